# revision 63
# baseline (speedup 1.0000x reference)
"""MASA agent-attention kernel for Trainium2, 8-core SPMD.

Sharding: core = (batch b in 0..3) x (head-group hg in 0..1).
Each core computes conv1x1 + depthwise3x3 for its 4 heads' q/k/v/a
channels (384 of 768), the agent attention for those heads, and SimAM
over its 96 output channels. No cross-core communication.

Per-core channel order: [q(96), k(96), a(0:64), v(96), a(64:96)].
SBUF slabs of 128: s0 = q[0:96]+k[0:32], s1 = k[32:96]+a[0:64],
s2 = v[0:96]+a[64:96].  v at slab base 0 so the v-transpose is one
[96,128] PE matmul (vs identity) per 128-pixel chunk.

Engine-op partition windows must be 32-aligned and (base==0 or count<=32).
"""

import sys
import types
import numpy as np

import concourse.bacc as bacc
import concourse.bass as bass
import concourse.mybir as mybir
from concourse.tile import TileContext
from concourse.bass_utils import run_bass_kernel_spmd

F16 = mybir.dt.float16
F32 = mybir.dt.float32
AX = mybir.AxisListType
OP = mybir.AluOpType
AF = mybir.ActivationFunctionType

B, C, H, W = 4, 192, 128, 128
N = H * W              # 16384
M_AG = 64              # agent tokens
E_LAMBDA = 1e-4
RS = 130               # padded row stride for pre
PREFREE = RS * RS      # 16900

TAPS = [(dy, dx) for dy in (-1, 0, 1) for dx in (-1, 0, 1)]
# tap offset in pre: (1+dy)*RS + (1+dx); odd offsets (dx==0) are
# 4B-misaligned for fp16 2x mode -> always on PE. DVE gets only the
# dx=+1 column (aligned), as tensor_scalar products + tensor_tensor
# adds (packed modes); scalar_tensor_tensor is always 1x on DVE.
PE_TAPS = {
    0: TAPS,                                  # slab0 fully on PE
    1: [t for t in TAPS if t[1] <= 0],        # dx in {-1, 0}
    2: [t for t in TAPS if t[1] <= 0],
}
DVE_TAPS = {s: [t for t in TAPS if t not in PE_TAPS[s]] for s in range(3)}
WDIAG_SLOT = {}
for _s in range(3):
    for _t in PE_TAPS[_s]:
        WDIAG_SLOT[(_s, _t[0], _t[1])] = len(WDIAG_SLOT)
NDIAG = len(WDIAG_SLOT)

NB2 = 16               # block count for norm / attention / simam phases
BLK2 = 1024
NCH = 128              # s-chunks of 128 for k-side


def _install_ntff_hook():
    try:
        import antenv.axon_hooks  # noqa: F401
        return
    except ImportError:
        pass
    try:
        from trn_agent_boot.trn_boot import _ntff_profile_via_ctypes
        hook = _ntff_profile_via_ctypes('/opt/axon/libaxon_pjrt.so')
        mod = types.ModuleType("antenv.axon_hooks")
        mod.get_axon_ntff_profile_hook = lambda: hook
        mod.set_axon_ntff_profile_hook = lambda h: None
        sys.modules["antenv.axon_hooks"] = mod
    except Exception:
        pass


def build_nc(debug=False):
    nc = bacc.Bacc("TRN2", target_bir_lowering=False, debug=False, num_devices=8)

    # ---- DRAM I/O ----
    xin = nc.dram_tensor("xin", [192, N], F16, kind="ExternalInput").ap()
    w1a = nc.dram_tensor("w1a", [96, 384], F16, kind="ExternalInput").ap()
    w1b = nc.dram_tensor("w1b", [96, 384], F16, kind="ExternalInput").ap()
    wdiag = nc.dram_tensor("wdiag", [128, NDIAG * 128], F16, kind="ExternalInput").ap()
    wtap = nc.dram_tensor("wtap", [128, 27], F32, kind="ExternalInput").ap()
    tmp0 = nc.dram_tensor("tmp0", [48, 1], F32, kind="ExternalInput").ap()
    tmp1 = nc.dram_tensor("tmp1", [48, 1], F32, kind="ExternalInput").ap()
    pat = nc.dram_tensor("pat", [128, 496], F16, kind="ExternalInput").ap()
    out_d = nc.dram_tensor("out", [96, N], F32, kind="ExternalOutput").ap()
    if debug:
        dbg_pre = nc.dram_tensor("dbg_pre", [128, PREFREE], F16, kind="ExternalOutput").ap()
        dbg_q = nc.dram_tensor("dbg_q", [128, N], F16, kind="ExternalOutput").ap()
        dbg_k = nc.dram_tensor("dbg_k", [128, N], F16, kind="ExternalOutput").ap()
        dbg_qn = nc.dram_tensor("dbg_qn", [128, N], F16, kind="ExternalOutput").ap()
        dbg_ag = nc.dram_tensor("dbg_ag", [96, 256], F16, kind="ExternalOutput").ap()
        dbg_av0 = nc.dram_tensor("dbg_av0", [128, 48], F16, kind="ExternalOutput").ap()
        dbg_av1 = nc.dram_tensor("dbg_av1", [128, 48], F16, kind="ExternalOutput").ap()
        dbg_xa = nc.dram_tensor("dbg_xa", [128, N], F16, kind="ExternalOutput").ap()
        dbg_vt = nc.dram_tensor("dbg_vt", [128, 98 * 4], F16, kind="ExternalOutput").ap()
        dbg_avi = nc.dram_tensor("dbg_avi", [128, 48], F16, kind="ExternalOutput").ap()
        dbg_e1 = nc.dram_tensor("dbg_e1", [128, BLK2], F16, kind="ExternalOutput").ap()
        dbg_op = nc.dram_tensor("dbg_op", [128, BLK2], F32, kind="ExternalOutput").ap()
        dbg_rqs = nc.dram_tensor("dbg_rqs", [48, BLK2], F32, kind="ExternalOutput").ap()

    # ---- persistent SBUF ----
    scratch = nc.alloc_sbuf_tensor("scratch", [128, PREFREE], F16).ap()
    dw0 = nc.alloc_sbuf_tensor("dw0", [128, N], F16).ap()
    dw1 = nc.alloc_sbuf_tensor("dw1", [128, N], F16).ap()
    dw2 = nc.alloc_sbuf_tensor("dw2", [128, N], F16).ap()
    dws = [dw0, dw1, dw2]
    w1a_s = nc.alloc_sbuf_tensor("w1a_s", [96, 384], F16).ap()
    w1b_s = nc.alloc_sbuf_tensor("w1b_s", [96, 384], F16).ap()
    wdiag_s = nc.alloc_sbuf_tensor("wdiag_s", [128, NDIAG * 128], F16).ap()
    wtap_s = nc.alloc_sbuf_tensor("wtap_s", [128, 27], F32).ap()
    ones_q = nc.alloc_sbuf_tensor("ones_q", [96, 96], F16).ap()
    ones_kA = nc.alloc_sbuf_tensor("ones_kA", [32, 128], F16).ap()
    ones_kB = nc.alloc_sbuf_tensor("ones_kB", [64, 128], F16).ap()
    ag_full = nc.alloc_sbuf_tensor("ag_full", [96, 256], F16).ap()
    agf = nc.alloc_sbuf_tensor("agf", [96, M_AG], F32).ap()
    agfs = nc.alloc_sbuf_tensor("agfs", [96, M_AG], F16).ap()
    temp_rep = nc.alloc_sbuf_tensor("temp_rep", [96, 1], F32).ap()
    av_l0 = nc.alloc_sbuf_tensor("av_l0", [128, 48], F16).ap()
    av_l1 = nc.alloc_sbuf_tensor("av_l1", [128, 48], F16).ap()
    dv_ones = nc.alloc_sbuf_tensor("dv_ones", [128, 48], F16).ap()
    idmat = nc.alloc_sbuf_tensor("idmat", [128, 128], F16).ap()
    asum = nc.alloc_sbuf_tensor("asum", [128, 2 * M_AG], F32).ap()  # rows 64:128
    as1_t = nc.alloc_sbuf_tensor("as1_t", [128, 1024], F32).ap()    # pool stage1
    rq2a = nc.alloc_sbuf_tensor("rq2a", [128, 1], F32).ap()
    rq2b = nc.alloc_sbuf_tensor("rq2b", [128, 1], F32).ap()
    mu_parts = nc.alloc_sbuf_tensor("mu_parts", [128, NB2], F32).ap()
    x2_parts = nc.alloc_sbuf_tensor("x2_parts", [128, NB2], F32).ap()
    musum = nc.alloc_sbuf_tensor("musum", [128, 1], F32).ap()
    sx2 = nc.alloc_sbuf_tensor("sx2", [128, 1], F32).ap()
    sden = nc.alloc_sbuf_tensor("sden", [128, 1], F32).ap()
    s_ch = nc.alloc_sbuf_tensor("s_ch", [128, 1], F32).ap()
    sqs = nc.alloc_sbuf_tensor("sqs", [128, 1], F32).ap()
    biasb = nc.alloc_sbuf_tensor("biasb", [128, 1], F32).ap()
    half_s = nc.alloc_sbuf_tensor("half_s", [128, 1], F32).ap()

    # aliases (sequential reuse of big buffers)
    pre3 = scratch.rearrange("p (y x) -> p y x", x=RS)   # padded conv out
    vT = scratch[:, 0:NCH * 98]                          # after dwconv
    x_attn = dw1[:, :]                                   # [128, N] f16 (phase D)
    kfull = dw2[0:96, :]                                 # k-hat packed (phase B)

    with TileContext(nc) as tc:
        with (
            tc.tile_pool(name="xio", bufs=4) as xio,
            tc.tile_pool(name="pout", bufs=2) as pout,
            tc.tile_pool(name="work", bufs=2) as work,
            tc.tile_pool(name="work1", bufs=1) as work1,
            tc.tile_pool(name="ppsum", bufs=2, space="PSUM") as ppsum,
        ):
            # ================= init =================
            nc.sync.dma_start(out=w1a_s[:], in_=w1a[:])
            nc.sync.dma_start(out=w1b_s[:], in_=w1b[:])
            nc.sync.dma_start(out=wdiag_s[:], in_=wdiag[:])
            nc.sync.dma_start(out=wtap_s[:], in_=wtap[:])
            # static patterns
            nc.sync.dma_start(out=ones_q[:], in_=pat[0:96, 0:96])
            nc.sync.dma_start(out=ones_kA[:], in_=pat[0:32, 368:496])
            nc.sync.dma_start(out=ones_kB[:], in_=pat[32:96, 368:496])
            nc.gpsimd.memset(av_l0[:], 0.0)
            nc.gpsimd.memset(av_l1[:], 0.0)
            # D1-rep ones lhsT: col j<24 -> even head (rows 0:64),
            # j>=24 -> odd head (rows 64:128)
            nc.sync.dma_start(out=dv_ones[:, 0:24], in_=pat[:, 192:216])
            nc.sync.dma_start(out=dv_ones[:, 24:48], in_=pat[:, 216:240])
            nc.sync.dma_start(out=idmat[:], in_=pat[:, 240:368])
            nc.gpsimd.memset(ag_full[:], 0.0)
            nc.sync.dma_start(out=temp_rep[0:48, :], in_=tmp0[:])
            nc.sync.dma_start(out=temp_rep[48:96, :], in_=tmp1[:])
            nc.gpsimd.memset(half_s[:], 0.5)
            # pre borders (rows 0 and 129, cols 0 and 129)
            nc.gpsimd.memset(pre3[:, 0, :], 0.0)
            nc.gpsimd.memset(pre3[:, 129, :], 0.0)
            nc.gpsimd.memset(pre3[:, :, 0], 0.0)
            nc.gpsimd.memset(pre3[:, :, 129], 0.0)

            if debug:
                nc.sync.dma_start(out=dbg_avi[:], in_=dv_ones[:])
            # ================= sweep1: conv1x1 + dwconv ====
            for s in range(3):
                wa = w1a_s[:, s * 128:(s + 1) * 128]
                wb = w1b_s[:, s * 128:(s + 1) * 128]
                nblk = N // 1024  # 16 blocks of 1024 (8 y-rows)

                def conv_blk(j, s=s, wa=wa, wb=wb):
                    x0 = xio.tile([96, 1024], F16, tag="x")
                    x1 = xio.tile([96, 1024], F16, tag="x")
                    nc.sync.dma_start(out=x0[:], in_=xin[0:96, j * 1024:(j + 1) * 1024])
                    nc.sync.dma_start(out=x1[:], in_=xin[96:192, j * 1024:(j + 1) * 1024])
                    ps = ppsum.tile([128, 1024], F32, tag="pA")
                    for q in range(2):
                        sl = slice(q * 512, (q + 1) * 512)
                        nc.tensor.matmul(ps[:, sl], wa, x0[:, sl], start=True, stop=False)
                        nc.tensor.matmul(ps[:, sl], wb, x1[:, sl], start=False, stop=True)
                    nc.scalar.copy(pre3[:, 1 + 8 * j: 9 + 8 * j, 1:129], ps[:])

                def dw_blk(j, s=s):
                    dst = dws[s][:, j * 1024:(j + 1) * 1024]
                    pe_t = PE_TAPS[s]
                    dv_t = DVE_TAPS[s]
                    pd = None
                    if pe_t:
                        pd = ppsum.tile([128, 1024], F32, tag="pB")
                        for q in range(2):
                            for ti, (dy, dx) in enumerate(pe_t):
                                dg = wdiag_s[:, WDIAG_SLOT[(s, dy, dx)] * 128:
                                             (WDIAG_SLOT[(s, dy, dx)] + 1) * 128]
                                rv = pre3[:, 1 + dy + 8 * j + 4 * q: 5 + dy + 8 * j + 4 * q,
                                          1 + dx: 129 + dx]
                                nc.tensor.matmul(pd[:, q * 512:(q + 1) * 512], dg, rv,
                                                 start=(ti == 0), stop=(ti == len(pe_t) - 1))
                    if dv_t:
                        # 3 aligned taps: STT on DVE (merges PE psum, 1x);
                        # the other 2 products on gpsimd (idle in sweep1),
                        # summed into dst by 2 DVE TT adds.
                        def win(dy, dx):
                            return pre3[:, 1 + dy + 8 * j: 9 + dy + 8 * j,
                                        1 + dx: 129 + dx]

                        def wsc(dy, dx):
                            ti = s * 9 + TAPS.index((dy, dx))
                            return wtap_s[:, ti:ti + 1]

                        ta = work.tile([128, 1024], F16, tag="dta")
                        nc.vector.scalar_tensor_tensor(
                            out=ta[:], in0=win(*dv_t[0]), scalar=wsc(*dv_t[0]),
                            in1=pd[:], op0=OP.mult, op1=OP.add)
                        tb = work.tile([128, 1024], F16, tag="dtb")
                        nc.gpsimd.tensor_scalar(
                            out=tb[:], in0=win(*dv_t[1]), scalar1=wsc(*dv_t[1]),
                            scalar2=None, op0=OP.mult)
                        nc.gpsimd.tensor_scalar(
                            out=dst, in0=win(*dv_t[2]), scalar1=wsc(*dv_t[2]),
                            scalar2=None, op0=OP.mult)
                        nc.vector.tensor_tensor(out=dst, in0=ta[:], in1=dst,
                                                op=OP.add)
                        nc.vector.tensor_tensor(out=dst, in0=tb[:], in1=dst,
                                                op=OP.add)
                    else:
                        nc.scalar.copy(dst, pd[:])

                conv_blk(0)
                for j in range(1, nblk):
                    conv_blk(j)
                    dw_blk(j - 1)
                dw_blk(nblk - 1)

                # pooling (both stages), emitted right after the slab that
                # produces its a-rows (as1_t is a dedicated buffer, so no
                # false dependency on the pre3 scratch region; rows 96:128
                # of as1_t are reused sequentially by the 2nd and 3rd group)
                if s == 1:
                    pgroups = ((dw1, 64, 0), (dw1, 96, 0))
                elif s == 2:
                    pgroups = ((dw2, 96, 1),)
                else:
                    pgroups = ()
                for (abuf, w0, half) in pgroups:
                    a3 = abuf[w0:w0 + 32, :].rearrange("p (a xi) -> p a xi",
                                                       xi=16)
                    s1 = as1_t[w0:w0 + 32, :]
                    nc.vector.reduce_sum(s1, a3, axis=AX.X)
                    as3 = s1.rearrange("p (yb yi xb) -> p yb xb yi",
                                       yb=8, yi=16, xb=8)
                    asum3 = asum[w0:w0 + 32,
                                 half * 64:(half + 1) * 64].rearrange(
                        "p (yb xb) -> p yb xb", yb=8)
                    nc.vector.reduce_sum(asum3, as3, axis=AX.X)

            if debug:
                nc.sync.dma_start(out=dbg_pre[:], in_=scratch[:])
                nc.sync.dma_start(out=dbg_q[:], in_=dw0[:])
                nc.sync.dma_start(out=dbg_k[:], in_=dw1[:])
            nc.sync.dma_start(out=agf[0:32, :], in_=asum[64:96, 0:64])
            nc.sync.dma_start(out=agf[32:64, :], in_=asum[96:128, 0:64])
            nc.sync.dma_start(out=agf[64:96, :], in_=asum[96:128, 64:128])
            # scale by temp/256 (per-partition scalar), then place blocks by DMA
            nc.vector.tensor_scalar(out=agfs[:], in0=agf[:],
                                    scalar1=temp_rep[:], scalar2=1.0 / 256.0,
                                    op0=OP.mult, op1=OP.mult)
            for h in range(4):
                nc.sync.dma_start(
                    out=ag_full[h * 24:(h + 1) * 24, h * 64:(h + 1) * 64],
                    in_=agfs[h * 24:(h + 1) * 24, :])

            # vT ones (denominator) columns; gpsimd queue is otherwise empty
            # here so these run as soon as the pre3 readers finish
            vT3 = vT.rearrange("p (c w) -> p c w", w=98)
            nc.gpsimd.memset(vT3[:, :, 0], 1.0)
            nc.gpsimd.memset(vT3[:, :, 97], 1.0)

            # ====== merged middle: per j: l2norm + vT group + k-side =====
            # ====== vT build: one dense PE burst =========================
            # fills the PE hole while pooling / sweep-tail drain the DVE.
            # Runs BEFORE any kpack DMA: kfull aliases dw2[0:96], so packing
            # k-hat destroys v.  (4 chunks per 2KB PSUM bank: a matmul
            # output must not cross a bank boundary.)
            for j in range(NB2):
                pt = ppsum.tile([128, 1024], F32, tag="pA", name="pt")
                for ci in range(8):
                    ssl = slice((8 * j + ci) * 128, (8 * j + ci + 1) * 128)
                    off = 512 * (ci // 4) + 96 * (ci % 4)
                    nc.tensor.matmul(pt[:, off:off + 96],
                                     dw2[0:96, ssl], idmat[0:96, 0:96],
                                     start=True, stop=True)
                pt3 = pt.rearrange("p (b x) -> p b x", b=2)
                nc.scalar.copy(vT3[:, 8 * j:8 * j + 8, 1:97],
                               pt3[:, :, 0:384])

            # keeps the PE queue dense through this region (HAM stays warm)
            for j in range(NB2):
                blk = slice(j * BLK2, (j + 1) * BLK2)
                sq0 = work1.tile([128, BLK2], F16, tag="sq0")
                sq1 = work1.tile([64, BLK2], F16, tag="sq1")
                sqk = work1.tile([32, BLK2], F16, tag="sqk")
                nc.gpsimd.tensor_tensor(out=sq0[:], in0=dw0[:, blk], in1=dw0[:, blk],
                                        op=OP.mult)
                nc.vector.tensor_tensor(out=sq1[:], in0=dw1[0:64, blk],
                                        in1=dw1[0:64, blk], op=OP.mult)
                nc.sync.dma_start(out=sqk[:], in_=sq0[96:128, :])
                pq = ppsum.tile([96, BLK2], F32, tag="pA")
                pk = ppsum.tile([128, BLK2], F32, tag="pB")
                for q in range(2):
                    sl = slice(q * 512, (q + 1) * 512)
                    nc.tensor.matmul(pq[:, sl], ones_q[:], sq0[0:96, sl],
                                     start=True, stop=True)
                    nc.tensor.matmul(pk[:, sl], ones_kA[:], sqk[:, sl],
                                     start=True, stop=False)
                    nc.tensor.matmul(pk[:, sl], ones_kB[:], sq1[:, sl],
                                     start=False, stop=True)
                rinv_q = work1.tile([96, BLK2], F16, tag="rinv_q")
                rinv_k = work1.tile([128, BLK2], F16, tag="rinv_k")
                nc.scalar.activation(rinv_q[:], pq[:], AF.Abs_reciprocal_sqrt)
                nc.scalar.activation(rinv_k[:], pk[:], AF.Abs_reciprocal_sqrt)
                nc.vector.tensor_tensor(out=dw0[0:96, blk], in0=dw0[0:96, blk],
                                        in1=rinv_q[:], op=OP.mult)
                nc.vector.tensor_tensor(out=dw0[96:128, blk], in0=dw0[96:128, blk],
                                        in1=rinv_k[96:128, :], op=OP.mult)
                nc.gpsimd.tensor_tensor(out=dw1[0:64, blk], in0=dw1[0:64, blk],
                                        in1=rinv_k[0:64, :], op=OP.mult)
                nc.sync.dma_start(out=kfull[0:32, blk], in_=dw0[96:128, blk])
                nc.sync.dma_start(out=kfull[32:96, blk], in_=dw1[0:64, blk])
            if debug:
                nc.sync.dma_start(out=dbg_qn[:], in_=dw0[:])

            # ====== k-side: l2 -> exp -> agvT accumulate (dense PE loop) ==
            # agvT[r, m] = sum_n vT3[n, r] * e2[n, m]: rows = [den|v|den],
            # cols = 256 agents; one 256-col matmul per chunk.
            agvTp = ppsum.tile([128, 256], F32, tag="pB", name="agvTp")
            NQ = NCH // 4

            def ks_l2exp(qq):
                l2p = ppsum.tile([128, 1024], F32, tag="pA", name="l2p")
                for ci in range(4):
                    ssl = slice((4 * qq + ci) * 128, (4 * qq + ci + 1) * 128)
                    nc.tensor.matmul(l2p[:, ci * 256:(ci + 1) * 256],
                                     kfull[:, ssl], ag_full[:],
                                     start=True, stop=True)
                e2t = work.tile([128, 1024], F16, tag="e2t", name="e2t")
                nc.scalar.activation(e2t[:], l2p[:], AF.Exp)
                return e2t

            # software-pipelined: l2(q+1) is emitted before agvT(q) so the
            # in-order PE queue never stalls on the exp of the current quad
            e_prev = ks_l2exp(0)
            for qq in range(NQ):
                e_next = ks_l2exp(qq + 1) if qq + 1 < NQ else None
                for ci in range(4):
                    nc.tensor.matmul(agvTp[0:98, :],
                                     vT3[:, 4 * qq + ci, :],
                                     e_prev[:, ci * 256:(ci + 1) * 256],
                                     start=(qq == 0 and ci == 0),
                                     stop=(qq == NQ - 1 and ci == 3))
                e_prev = e_next

            # ====== av_l build: f16-convert agvT, transpose per head-pair =
            agvT_f16 = work1.tile([128, 256], F16, tag="agvf")
            nc.scalar.copy(agvT_f16[:], agvTp[:])
            avT0 = ppsum.tile([128, 128], F32, tag="pA", name="avT0")
            avT1 = ppsum.tile([128, 128], F32, tag="pB", name="avT1")
            nc.tensor.matmul(avT0[:, 0:98], agvT_f16[0:98, 0:128],
                             idmat[0:98, 0:98], start=True, stop=True)
            nc.tensor.matmul(avT1[:, 0:98], agvT_f16[0:98, 128:256],
                             idmat[0:98, 0:98], start=True, stop=True)
            # avT*: rows = agents, col 0 = denominator, cols 1:97 = v-ch 0:96
            nc.vector.reciprocal_approx_fast(out=rq2a[:], in_=avT0[:, 0:1])
            nc.vector.reciprocal_approx_fast(out=rq2b[:], in_=avT1[:, 0:1])
            # block-diagonal: even head of pair -> rows 0:64 x cols 0:24,
            # odd head -> rows 64:128 x cols 24:48 (other entries stay zero)
            nc.vector.tensor_scalar(out=av_l0[0:64, 0:24], in0=avT0[0:64, 1:25],
                                    scalar1=rq2a[0:64, :], scalar2=None, op0=OP.mult)
            for w0 in (64, 96):
                nc.vector.tensor_scalar(out=av_l0[w0:w0 + 32, 24:48],
                                        in0=avT0[w0:w0 + 32, 25:49],
                                        scalar1=rq2a[w0:w0 + 32, :], scalar2=None,
                                        op0=OP.mult)
            nc.vector.tensor_scalar(out=av_l1[0:64, 0:24], in0=avT1[0:64, 49:73],
                                    scalar1=rq2b[0:64, :], scalar2=None, op0=OP.mult)
            for w0 in (64, 96):
                nc.vector.tensor_scalar(out=av_l1[w0:w0 + 32, 24:48],
                                        in0=avT1[w0:w0 + 32, 73:97],
                                        scalar1=rq2b[w0:w0 + 32, :], scalar2=None,
                                        op0=OP.mult)

            if debug:
                nc.sync.dma_start(out=dbg_ag[:], in_=ag_full[:])
                nc.sync.dma_start(out=dbg_av0[:], in_=av_l0[:])
                nc.sync.dma_start(out=dbg_av1[:], in_=av_l1[:])
                nc.sync.dma_start(out=dbg_vt[:], in_=vT[:, 0:98 * 4])
            # ================= q-side + division =========================
            # Both head-pairs per j-block: op_/od_ psum rows 0:48 (hp0) and
            # 64:112 (hp1); one recip + one STT over [128, BLK2] covers both.
            # x_attn rows 48:64 / 112:128 are junk, skipped at output DMA.
            def qs_l1exp(j):
                e1s = []
                for hp in range(2):
                    ag_cols = ag_full[:, hp * 128:(hp + 1) * 128]
                    l1 = ppsum.tile([128, BLK2], F32, tag="pA", name="l1")
                    for q in range(2):
                        sl = slice(j * BLK2 + q * 512, j * BLK2 + (q + 1) * 512)
                        psl = slice(q * 512, (q + 1) * 512)
                        nc.tensor.matmul(l1[:, psl], ag_cols, dw0[0:96, sl],
                                         start=True, stop=True)
                    e1 = work.tile([128, BLK2], F16, tag=f"e1{hp}", name="e1")
                    nc.scalar.activation(e1[:], l1[:], AF.Exp)
                    e1s.append(e1)
                return e1s

            def qs_opod(j, e1s):
                blk = slice(j * BLK2, (j + 1) * BLK2)
                op_ = ppsum.tile([128, BLK2], F32, tag="pB", name="op_")
                od_ = ppsum.tile([128, BLK2], F32, tag="pB", name="od_")
                for hp in range(2):
                    rb = 64 * hp
                    av_l = av_l0 if hp == 0 else av_l1
                    for q in range(2):
                        psl = slice(q * 512, (q + 1) * 512)
                        nc.tensor.matmul(op_[rb:rb + 48, psl], av_l[:],
                                         e1s[hp][:, psl], start=True, stop=True)
                        nc.tensor.matmul(od_[rb:rb + 48, psl], dv_ones[:],
                                         e1s[hp][:, psl], start=True, stop=True)
                rqs = work1.tile([128, BLK2], F32, tag="rqs")
                nc.vector.reciprocal_approx_fast(out=rqs[:], in_=od_[:])
                nc.vector.scalar_tensor_tensor(
                    out=x_attn[:, blk], in0=op_[:], scalar=0.0,
                    in1=rqs[:], op0=OP.bypass, op1=OP.mult,
                    accum_out=mu_parts[:, j:j + 1])
                x2t = work.tile([128, BLK2], F16, tag="x2t")
                nc.scalar.activation(x2t[:], x_attn[:, blk], AF.Square,
                                     accum_out=x2_parts[:, j:j + 1])

            # software-pipelined like the k-side
            pend = qs_l1exp(0)
            for j in range(1, NB2):
                nxt = qs_l1exp(j)
                qs_opod(j - 1, pend)
                pend = nxt
            qs_opod(NB2 - 1, pend)

            if debug:
                nc.sync.dma_start(out=dbg_xa[:], in_=x_attn[:])
            # ================= SimAM =====================================
            # all [128, *]: rows 48:64 / 112:128 are junk lanes, skipped at
            # the output DMAs; per-partition stats keep junk contained.
            # sum(d2) = sum(x^2) - N*mu^2 (both accumulated in the q-side),
            # and s*(x-mu)^2 = (sqrt(s)*x - sqrt(s)*mu)^2 folds into one
            # Square activation, so no separate d2 pass over N is needed.
            nc.vector.reduce_sum(musum[:], mu_parts[:], axis=AX.X)
            nc.vector.reduce_sum(sx2[:], x2_parts[:], axis=AX.X)
            mu2 = work1.tile([128, 1], F32, tag="mu2")
            nc.vector.tensor_tensor(out=mu2[:], in0=musum[:], in1=musum[:],
                                    op=OP.mult)
            nc.vector.scalar_tensor_tensor(
                out=sden[:], in0=mu2[:], scalar=-1.0 / N, in1=sx2[:],
                op0=OP.mult, op1=OP.add)
            nc.vector.tensor_scalar(out=sden[:], in0=sden[:],
                                    scalar1=4.0 / (N - 1), scalar2=4.0 * E_LAMBDA,
                                    op0=OP.mult, op1=OP.add)
            nc.vector.reciprocal_approx_fast(out=s_ch[:], in_=sden[:])
            nc.scalar.activation(sqs[:], s_ch[:], AF.Sqrt)
            nc.vector.tensor_scalar(out=biasb[:], in0=musum[:], scalar1=sqs[:],
                                    scalar2=-1.0 / N, op0=OP.mult, op1=OP.mult)
            for j in range(NB2):
                blk = slice(j * BLK2, (j + 1) * BLK2)
                d2t = work.tile([128, BLK2], F16, tag="d2t")
                nc.scalar.activation(d2t[:], x_attn[:, blk], AF.Square,
                                     bias=biasb[:], scale=sqs[:])
                sig_t = work.tile([128, BLK2], F16, tag="sig_t")
                nc.scalar.activation(sig_t[:], d2t[:], AF.Sigmoid,
                                     bias=half_s[:])
                ob = pout.tile([128, BLK2], F32, tag="ob")
                nc.vector.tensor_tensor(out=ob[:], in0=x_attn[:, blk],
                                        in1=sig_t[:], op=OP.mult)
                nc.sync.dma_start(out=out_d[0:48, blk], in_=ob[0:48, :])
                nc.sync.dma_start(out=out_d[48:96, blk], in_=ob[64:112, :])

    nc.compile()
    return nc


_NC = None


def _get_nc():
    global _NC
    if _NC is None:
        _install_ntff_hook()
        _NC = build_nc()
    return _NC


def make_core_inputs(x, w_qkv, w_dw, temperature):
    """Host-side shard prep. Returns list of 8 input dicts."""
    x = np.asarray(x)
    w_qkv = np.asarray(w_qkv)
    w_dw = np.asarray(w_dw)
    temperature = np.asarray(temperature).reshape(8)
    in_maps = []
    for core in range(8):
        b, hg = core // 2, core % 2
        # slab0 = q + k[0:32]; slab1 = k[32:96] + a[0:64];
        # slab2 = v[0:96] + a[64:96]  (v at base 0 for PE transpose)
        rows = np.concatenate([
            np.arange(hg * 96, hg * 96 + 96),           # q
            192 + np.arange(hg * 96, hg * 96 + 96),     # k
            576 + np.arange(hg * 96, hg * 96 + 64),     # a[0:64]
            384 + np.arange(hg * 96, hg * 96 + 96),     # v
            576 + np.arange(hg * 96 + 64, hg * 96 + 96),  # a[64:96]
        ])
        W1 = w_qkv[rows, :, 0, 0]                        # [384, 192]
        W1T = np.ascontiguousarray(W1.T).astype(np.float16)
        wd9 = w_dw[rows, 0].reshape(384, 9).astype(np.float32)
        wdiag_h = np.zeros((128, NDIAG * 128), np.float16)
        wtap_h = np.zeros((128, 27), np.float32)
        for s in range(3):
            for t in range(9):
                wtap_h[:, s * 9 + t] = wd9[s * 128:(s + 1) * 128, t]
        for (s, dy, dx), idx in WDIAG_SLOT.items():
            t = (dy + 1) * 3 + (dx + 1)
            wdiag_h[np.arange(128), idx * 128 + np.arange(128)] = \
                wd9[s * 128:(s + 1) * 128, t].astype(np.float16)
        pat_h = np.zeros((128, 496), np.float16)
        pat_h[np.arange(128), 240 + np.arange(128)] = 1  # I128 for transposes
        for h in range(4):
            pat_h[h * 24:(h + 1) * 24, h * 24:(h + 1) * 24] = 1    # ones_q
        # D1-rep ones: cols 192:216 (rows 0:64), cols 216:240 (rows 64:128)
        pat_h[0:64, 192:216] = 1
        pat_h[64:128, 216:240] = 1
        # ones_kA/ones_kB at cols 368:496: pk output row m maps directly to
        # the k-channel partition homes: rows 0:64 -> k-ch 32+m (dw1),
        # rows 96:128 -> k-ch m-96 (dw0); rows 64:96 unused.
        hrow = np.full(128, -1)
        hrow[0:64] = (32 + np.arange(64)) // 24
        hrow[96:128] = np.arange(32) // 24
        pat_h[0:32, 368:496] = (
            (np.arange(32)[:, None] // 24) == hrow[None, :]).astype(np.float16)
        pat_h[32:96, 368:496] = (
            ((32 + np.arange(64))[:, None] // 24) == hrow[None, :]
        ).astype(np.float16)
        heads = np.arange(hg * 4, hg * 4 + 4)
        t4 = temperature[heads].astype(np.float32)
        in_maps.append({
            "xin": x[b].reshape(192, N).astype(np.float16),
            "w1a": W1T[0:96].copy(),
            "w1b": W1T[96:192].copy(),
            "wdiag": wdiag_h,
            "wtap": wtap_h,
            "tmp0": np.repeat(t4[0:2], 24).reshape(48, 1).copy(),
            "tmp1": np.repeat(t4[2:4], 24).reshape(48, 1).copy(),
            "pat": pat_h,
        })
    return in_maps


def _assemble(results):
    full = np.empty((B, C, H, W), np.float32)
    for core in range(8):
        b, hg = core // 2, core % 2
        full[b, hg * 96:(hg + 1) * 96] = results[core]["out"].reshape(96, H, W)
    return full


def kernel(x, w_qkv, w_dw, temperature):
    nc = _get_nc()
    in_maps = make_core_inputs(x, w_qkv, w_dw, temperature)
    res = run_bass_kernel_spmd(nc, in_maps, list(range(8)))
    return _assemble(res.results)


def kernel_profiled(x, w_qkv, w_dw, temperature):
    nc = _get_nc()
    in_maps = make_core_inputs(x, w_qkv, w_dw, temperature)
    res = run_bass_kernel_spmd(nc, in_maps, list(range(8)), trace=True)
    return _assemble(res.results), res.exec_time_ns



# revision 65
# speedup vs baseline: 2.6263x; 2.6263x over previous
"""MASA agent-attention kernel for Trainium2, 8-core SPMD.

Sharding: core = (batch b in 0..3) x (head-group hg in 0..1).
Each core computes conv1x1 + depthwise3x3 for its 4 heads' q/k/v/a
channels (384 of 768), the agent attention for those heads, and SimAM
over its 96 output channels. No cross-core communication.

Per-core channel order: [q(96), k(96), a(0:64), v(96), a(64:96)].
SBUF slabs of 128: s0 = q[0:96]+k[0:32], s1 = k[32:96]+a[0:64],
s2 = v[0:96]+a[64:96].  v at slab base 0 so the v-transpose is one
[96,128] PE matmul (vs identity) per 128-pixel chunk.

Engine-op partition windows must be 32-aligned and (base==0 or count<=32).
"""

import sys
import types
import numpy as np

import concourse.bacc as bacc
import concourse.bass as bass
import concourse.mybir as mybir
from concourse.tile import TileContext
from concourse.bass_utils import run_bass_kernel_spmd

F16 = mybir.dt.float16
F32 = mybir.dt.float32
AX = mybir.AxisListType
OP = mybir.AluOpType
AF = mybir.ActivationFunctionType

B, C, H, W = 4, 192, 128, 128
N = H * W              # 16384
M_AG = 64              # agent tokens
E_LAMBDA = 1e-4
RS = 130               # padded row stride for pre
PREFREE = RS * RS      # 16900

TAPS = [(dy, dx) for dy in (-1, 0, 1) for dx in (-1, 0, 1)]
# tap offset in pre: (1+dy)*RS + (1+dx); odd offsets (dx==0) are
# 4B-misaligned for fp16 2x mode -> always on PE. DVE gets only the
# dx=+1 column (aligned), as tensor_scalar products + tensor_tensor
# adds (packed modes); scalar_tensor_tensor is always 1x on DVE.
PE_TAPS = {
    0: TAPS,                                  # slab0 fully on PE
    1: [t for t in TAPS if t[1] <= 0],        # dx in {-1, 0}
    2: [t for t in TAPS if t[1] <= 0],
}
DVE_TAPS = {s: [t for t in TAPS if t not in PE_TAPS[s]] for s in range(3)}
WDIAG_SLOT = {}
for _s in range(3):
    for _t in PE_TAPS[_s]:
        WDIAG_SLOT[(_s, _t[0], _t[1])] = len(WDIAG_SLOT)
NDIAG = len(WDIAG_SLOT)

NB2 = 16               # block count for norm / attention / simam phases
BLK2 = 1024
NCH = 128              # s-chunks of 128 for k-side


def _install_ntff_hook():
    try:
        import antenv.axon_hooks  # noqa: F401
        return
    except ImportError:
        pass
    try:
        from trn_agent_boot.trn_boot import _ntff_profile_via_ctypes
        hook = _ntff_profile_via_ctypes('/opt/axon/libaxon_pjrt.so')
        mod = types.ModuleType("antenv.axon_hooks")
        mod.get_axon_ntff_profile_hook = lambda: hook
        mod.set_axon_ntff_profile_hook = lambda h: None
        sys.modules["antenv.axon_hooks"] = mod
    except Exception:
        pass


def build_nc(debug=False):
    nc = bacc.Bacc("TRN2", target_bir_lowering=False, debug=False, num_devices=8)

    # ---- DRAM I/O ----
    xin = nc.dram_tensor("xin", [192, N], F16, kind="ExternalInput").ap()
    w1a = nc.dram_tensor("w1a", [96, 384], F16, kind="ExternalInput").ap()
    w1b = nc.dram_tensor("w1b", [96, 384], F16, kind="ExternalInput").ap()
    wdiag = nc.dram_tensor("wdiag", [128, NDIAG * 128], F16, kind="ExternalInput").ap()
    wtap = nc.dram_tensor("wtap", [128, 27], F32, kind="ExternalInput").ap()
    tmp0 = nc.dram_tensor("tmp0", [48, 1], F32, kind="ExternalInput").ap()
    tmp1 = nc.dram_tensor("tmp1", [48, 1], F32, kind="ExternalInput").ap()
    pat = nc.dram_tensor("pat", [128, 496], F16, kind="ExternalInput").ap()
    out_d = nc.dram_tensor("out", [96, N], F32, kind="ExternalOutput").ap()
    if debug:
        dbg_pre = nc.dram_tensor("dbg_pre", [128, PREFREE], F16, kind="ExternalOutput").ap()
        dbg_q = nc.dram_tensor("dbg_q", [128, N], F16, kind="ExternalOutput").ap()
        dbg_k = nc.dram_tensor("dbg_k", [128, N], F16, kind="ExternalOutput").ap()
        dbg_qn = nc.dram_tensor("dbg_qn", [128, N], F16, kind="ExternalOutput").ap()
        dbg_ag = nc.dram_tensor("dbg_ag", [96, 256], F16, kind="ExternalOutput").ap()
        dbg_av0 = nc.dram_tensor("dbg_av0", [128, 48], F16, kind="ExternalOutput").ap()
        dbg_av1 = nc.dram_tensor("dbg_av1", [128, 48], F16, kind="ExternalOutput").ap()
        dbg_xa = nc.dram_tensor("dbg_xa", [128, N], F16, kind="ExternalOutput").ap()
        dbg_vt = nc.dram_tensor("dbg_vt", [128, 98 * 4], F16, kind="ExternalOutput").ap()
        dbg_avi = nc.dram_tensor("dbg_avi", [128, 48], F16, kind="ExternalOutput").ap()
        dbg_e1 = nc.dram_tensor("dbg_e1", [128, BLK2], F16, kind="ExternalOutput").ap()
        dbg_op = nc.dram_tensor("dbg_op", [128, BLK2], F32, kind="ExternalOutput").ap()
        dbg_rqs = nc.dram_tensor("dbg_rqs", [48, BLK2], F32, kind="ExternalOutput").ap()

    # ---- persistent SBUF ----
    scratch = nc.alloc_sbuf_tensor("scratch", [128, PREFREE], F16).ap()
    dw0 = nc.alloc_sbuf_tensor("dw0", [128, N], F16).ap()
    dw1 = nc.alloc_sbuf_tensor("dw1", [128, N], F16).ap()
    dw2 = nc.alloc_sbuf_tensor("dw2", [128, N], F16).ap()
    dws = [dw0, dw1, dw2]
    w1a_s = nc.alloc_sbuf_tensor("w1a_s", [96, 384], F16).ap()
    w1b_s = nc.alloc_sbuf_tensor("w1b_s", [96, 384], F16).ap()
    wdiag_s = nc.alloc_sbuf_tensor("wdiag_s", [128, NDIAG * 128], F16).ap()
    wtap_s = nc.alloc_sbuf_tensor("wtap_s", [128, 27], F32).ap()
    ones_q = nc.alloc_sbuf_tensor("ones_q", [96, 96], F16).ap()
    ones_kA = nc.alloc_sbuf_tensor("ones_kA", [32, 128], F16).ap()
    ones_kB = nc.alloc_sbuf_tensor("ones_kB", [64, 128], F16).ap()
    ag_full = nc.alloc_sbuf_tensor("ag_full", [96, 256], F16).ap()
    agf = nc.alloc_sbuf_tensor("agf", [96, M_AG], F32).ap()
    agfs = nc.alloc_sbuf_tensor("agfs", [96, M_AG], F16).ap()
    temp_rep = nc.alloc_sbuf_tensor("temp_rep", [96, 1], F32).ap()
    av_l0 = nc.alloc_sbuf_tensor("av_l0", [128, 48], F16).ap()
    av_l1 = nc.alloc_sbuf_tensor("av_l1", [128, 48], F16).ap()
    dv_ones = nc.alloc_sbuf_tensor("dv_ones", [128, 48], F16).ap()
    idmat = nc.alloc_sbuf_tensor("idmat", [128, 128], F16).ap()
    asum = nc.alloc_sbuf_tensor("asum", [128, 2 * M_AG], F32).ap()  # rows 64:128
    as1_t = nc.alloc_sbuf_tensor("as1_t", [128, 1024], F32).ap()    # pool stage1
    rq2a = nc.alloc_sbuf_tensor("rq2a", [128, 1], F32).ap()
    rq2b = nc.alloc_sbuf_tensor("rq2b", [128, 1], F32).ap()
    mu_parts = nc.alloc_sbuf_tensor("mu_parts", [128, NB2], F32).ap()
    x2_parts = nc.alloc_sbuf_tensor("x2_parts", [128, NB2], F32).ap()
    musum = nc.alloc_sbuf_tensor("musum", [128, 1], F32).ap()
    sx2 = nc.alloc_sbuf_tensor("sx2", [128, 1], F32).ap()
    sden = nc.alloc_sbuf_tensor("sden", [128, 1], F32).ap()
    s_ch = nc.alloc_sbuf_tensor("s_ch", [128, 1], F32).ap()
    sqs = nc.alloc_sbuf_tensor("sqs", [128, 1], F32).ap()
    biasb = nc.alloc_sbuf_tensor("biasb", [128, 1], F32).ap()
    half_s = nc.alloc_sbuf_tensor("half_s", [128, 1], F32).ap()

    # aliases (sequential reuse of big buffers)
    pre3 = scratch.rearrange("p (y x) -> p y x", x=RS)   # padded conv out
    vT = scratch[:, 0:NCH * 98]                          # after dwconv
    x_attn = dw1[:, :]                                   # [128, N] f16 (phase D)
    kfull = dw2[0:96, :]                                 # k-hat packed (phase B)

    with TileContext(nc) as tc:
        with (
            tc.tile_pool(name="xio", bufs=4) as xio,
            tc.tile_pool(name="pout", bufs=2) as pout,
            tc.tile_pool(name="work", bufs=2) as work,
            tc.tile_pool(name="work1", bufs=1) as work1,
            tc.tile_pool(name="ppsum", bufs=2, space="PSUM") as ppsum,
        ):
            # ================= init =================
            nc.sync.dma_start(out=w1a_s[:], in_=w1a[:])
            nc.sync.dma_start(out=w1b_s[:], in_=w1b[:])
            nc.sync.dma_start(out=wdiag_s[:], in_=wdiag[:])
            nc.sync.dma_start(out=wtap_s[:], in_=wtap[:])
            # static patterns
            nc.sync.dma_start(out=ones_q[:], in_=pat[0:96, 0:96])
            nc.sync.dma_start(out=ones_kA[:], in_=pat[0:32, 368:496])
            nc.sync.dma_start(out=ones_kB[:], in_=pat[32:96, 368:496])
            nc.gpsimd.memset(av_l0[:], 0.0)
            nc.gpsimd.memset(av_l1[:], 0.0)
            # D1-rep ones lhsT: col j<24 -> even head (rows 0:64),
            # j>=24 -> odd head (rows 64:128)
            nc.sync.dma_start(out=dv_ones[:, 0:24], in_=pat[:, 192:216])
            nc.sync.dma_start(out=dv_ones[:, 24:48], in_=pat[:, 216:240])
            nc.sync.dma_start(out=idmat[:], in_=pat[:, 240:368])
            nc.gpsimd.memset(ag_full[:], 0.0)
            nc.sync.dma_start(out=temp_rep[0:48, :], in_=tmp0[:])
            nc.sync.dma_start(out=temp_rep[48:96, :], in_=tmp1[:])
            nc.gpsimd.memset(half_s[:], 0.5)
            # pre borders (rows 0 and 129, cols 0 and 129)
            nc.gpsimd.memset(pre3[:, 0, :], 0.0)
            nc.gpsimd.memset(pre3[:, 129, :], 0.0)
            nc.gpsimd.memset(pre3[:, :, 0], 0.0)
            nc.gpsimd.memset(pre3[:, :, 129], 0.0)

            if debug:
                nc.sync.dma_start(out=dbg_avi[:], in_=dv_ones[:])
            # ================= sweep1: conv1x1 + dwconv ====
            for s in range(3):
                wa = w1a_s[:, s * 128:(s + 1) * 128]
                wb = w1b_s[:, s * 128:(s + 1) * 128]
                nblk = N // 1024  # 16 blocks of 1024 (8 y-rows)

                def conv_blk(j, s=s, wa=wa, wb=wb):
                    x0 = xio.tile([96, 1024], F16, tag="x")
                    x1 = xio.tile([96, 1024], F16, tag="x")
                    nc.sync.dma_start(out=x0[:], in_=xin[0:96, j * 1024:(j + 1) * 1024])
                    nc.sync.dma_start(out=x1[:], in_=xin[96:192, j * 1024:(j + 1) * 1024])
                    ps = ppsum.tile([128, 1024], F32, tag="pA")
                    for q in range(2):
                        sl = slice(q * 512, (q + 1) * 512)
                        nc.tensor.matmul(ps[:, sl], wa, x0[:, sl], start=True, stop=False)
                        nc.tensor.matmul(ps[:, sl], wb, x1[:, sl], start=False, stop=True)
                    nc.scalar.copy(pre3[:, 1 + 8 * j: 9 + 8 * j, 1:129], ps[:])

                def dw_blk(j, s=s):
                    dst = dws[s][:, j * 1024:(j + 1) * 1024]
                    pe_t = PE_TAPS[s]
                    dv_t = DVE_TAPS[s]
                    pd = None
                    if pe_t:
                        pd = ppsum.tile([128, 1024], F32, tag="pB")
                        for q in range(2):
                            for ti, (dy, dx) in enumerate(pe_t):
                                dg = wdiag_s[:, WDIAG_SLOT[(s, dy, dx)] * 128:
                                             (WDIAG_SLOT[(s, dy, dx)] + 1) * 128]
                                rv = pre3[:, 1 + dy + 8 * j + 4 * q: 5 + dy + 8 * j + 4 * q,
                                          1 + dx: 129 + dx]
                                nc.tensor.matmul(pd[:, q * 512:(q + 1) * 512], dg, rv,
                                                 start=(ti == 0), stop=(ti == len(pe_t) - 1))
                    if dv_t:
                        # 3 aligned taps: STT on DVE (merges PE psum, 1x);
                        # the other 2 products on gpsimd (idle in sweep1),
                        # summed into dst by 2 DVE TT adds.
                        def win(dy, dx):
                            return pre3[:, 1 + dy + 8 * j: 9 + dy + 8 * j,
                                        1 + dx: 129 + dx]

                        def wsc(dy, dx):
                            ti = s * 9 + TAPS.index((dy, dx))
                            return wtap_s[:, ti:ti + 1]

                        ta = work.tile([128, 1024], F16, tag="dta")
                        nc.vector.scalar_tensor_tensor(
                            out=ta[:], in0=win(*dv_t[0]), scalar=wsc(*dv_t[0]),
                            in1=pd[:], op0=OP.mult, op1=OP.add)
                        tb = work.tile([128, 1024], F16, tag="dtb")
                        nc.vector.tensor_scalar(
                            out=tb[:], in0=win(*dv_t[1]), scalar1=wsc(*dv_t[1]),
                            scalar2=None, op0=OP.mult)
                        nc.vector.tensor_scalar(
                            out=dst, in0=win(*dv_t[2]), scalar1=wsc(*dv_t[2]),
                            scalar2=None, op0=OP.mult)
                        nc.vector.tensor_tensor(out=dst, in0=ta[:], in1=dst,
                                                op=OP.add)
                        nc.vector.tensor_tensor(out=dst, in0=tb[:], in1=dst,
                                                op=OP.add)
                    else:
                        nc.scalar.copy(dst, pd[:])

                conv_blk(0)
                for j in range(1, nblk):
                    conv_blk(j)
                    dw_blk(j - 1)
                dw_blk(nblk - 1)

                # pooling (both stages), emitted right after the slab that
                # produces its a-rows (as1_t is a dedicated buffer, so no
                # false dependency on the pre3 scratch region; rows 96:128
                # of as1_t are reused sequentially by the 2nd and 3rd group)
                if s == 1:
                    pgroups = ((dw1, 64, 0), (dw1, 96, 0))
                elif s == 2:
                    pgroups = ((dw2, 96, 1),)
                else:
                    pgroups = ()
                for (abuf, w0, half) in pgroups:
                    a3 = abuf[w0:w0 + 32, :].rearrange("p (a xi) -> p a xi",
                                                       xi=16)
                    s1 = as1_t[w0:w0 + 32, :]
                    nc.vector.reduce_sum(s1, a3, axis=AX.X)
                    as3 = s1.rearrange("p (yb yi xb) -> p yb xb yi",
                                       yb=8, yi=16, xb=8)
                    asum3 = asum[w0:w0 + 32,
                                 half * 64:(half + 1) * 64].rearrange(
                        "p (yb xb) -> p yb xb", yb=8)
                    nc.vector.reduce_sum(asum3, as3, axis=AX.X)

            if debug:
                nc.sync.dma_start(out=dbg_pre[:], in_=scratch[:])
                nc.sync.dma_start(out=dbg_q[:], in_=dw0[:])
                nc.sync.dma_start(out=dbg_k[:], in_=dw1[:])
            nc.sync.dma_start(out=agf[0:32, :], in_=asum[64:96, 0:64])
            nc.sync.dma_start(out=agf[32:64, :], in_=asum[96:128, 0:64])
            nc.sync.dma_start(out=agf[64:96, :], in_=asum[96:128, 64:128])
            # scale by temp/256 (per-partition scalar), then place blocks by DMA
            nc.vector.tensor_scalar(out=agfs[:], in0=agf[:],
                                    scalar1=temp_rep[:], scalar2=1.0 / 256.0,
                                    op0=OP.mult, op1=OP.mult)
            for h in range(4):
                nc.sync.dma_start(
                    out=ag_full[h * 24:(h + 1) * 24, h * 64:(h + 1) * 64],
                    in_=agfs[h * 24:(h + 1) * 24, :])

            # vT ones (denominator) columns; gpsimd queue is otherwise empty
            # here so these run as soon as the pre3 readers finish
            vT3 = vT.rearrange("p (c w) -> p c w", w=98)
            nc.gpsimd.memset(vT3[:, :, 0], 1.0)
            nc.gpsimd.memset(vT3[:, :, 97], 1.0)

            # ====== merged middle: per j: l2norm + vT group + k-side =====
            # ====== vT build: one dense PE burst =========================
            # fills the PE hole while pooling / sweep-tail drain the DVE.
            # Runs BEFORE any kpack DMA: kfull aliases dw2[0:96], so packing
            # k-hat destroys v.  (4 chunks per 2KB PSUM bank: a matmul
            # output must not cross a bank boundary.)
            for j in range(NB2):
                pt = ppsum.tile([128, 1024], F32, tag="pA", name="pt")
                for ci in range(8):
                    ssl = slice((8 * j + ci) * 128, (8 * j + ci + 1) * 128)
                    off = 512 * (ci // 4) + 96 * (ci % 4)
                    nc.tensor.matmul(pt[:, off:off + 96],
                                     dw2[0:96, ssl], idmat[0:96, 0:96],
                                     start=True, stop=True)
                pt3 = pt.rearrange("p (b x) -> p b x", b=2)
                nc.scalar.copy(vT3[:, 8 * j:8 * j + 8, 1:97],
                               pt3[:, :, 0:384])

            # keeps the PE queue dense through this region (HAM stays warm)
            for j in range(NB2):
                blk = slice(j * BLK2, (j + 1) * BLK2)
                sq0 = work1.tile([128, BLK2], F16, tag="sq0")
                sq1 = work1.tile([64, BLK2], F16, tag="sq1")
                sqk = work1.tile([32, BLK2], F16, tag="sqk")
                nc.gpsimd.tensor_tensor(out=sq0[:], in0=dw0[:, blk], in1=dw0[:, blk],
                                        op=OP.mult)
                nc.scalar.activation(sq1[:], dw1[0:64, blk], AF.Square)
                nc.sync.dma_start(out=sqk[:], in_=sq0[96:128, :])
                pq = ppsum.tile([96, BLK2], F32, tag="pA")
                pk = ppsum.tile([128, BLK2], F32, tag="pB")
                for q in range(2):
                    sl = slice(q * 512, (q + 1) * 512)
                    nc.tensor.matmul(pq[:, sl], ones_q[:], sq0[0:96, sl],
                                     start=True, stop=True)
                    nc.tensor.matmul(pk[:, sl], ones_kA[:], sqk[:, sl],
                                     start=True, stop=False)
                    nc.tensor.matmul(pk[:, sl], ones_kB[:], sq1[:, sl],
                                     start=False, stop=True)
                rinv_q = work1.tile([96, BLK2], F16, tag="rinv_q")
                rinv_k = work1.tile([128, BLK2], F16, tag="rinv_k")
                nc.scalar.activation(rinv_q[:], pq[:], AF.Abs_reciprocal_sqrt)
                nc.scalar.activation(rinv_k[:], pk[:], AF.Abs_reciprocal_sqrt)
                nc.vector.tensor_tensor(out=dw0[0:96, blk], in0=dw0[0:96, blk],
                                        in1=rinv_q[:], op=OP.mult)
                nc.vector.tensor_tensor(out=dw0[96:128, blk], in0=dw0[96:128, blk],
                                        in1=rinv_k[96:128, :], op=OP.mult)
                nc.gpsimd.tensor_tensor(out=dw1[0:64, blk], in0=dw1[0:64, blk],
                                        in1=rinv_k[0:64, :], op=OP.mult)
                nc.sync.dma_start(out=kfull[0:32, blk], in_=dw0[96:128, blk])
                nc.sync.dma_start(out=kfull[32:96, blk], in_=dw1[0:64, blk])
            if debug:
                nc.sync.dma_start(out=dbg_qn[:], in_=dw0[:])

            # ====== k-side: l2 -> exp -> agvT accumulate (dense PE loop) ==
            # agvT[r, m] = sum_n vT3[n, r] * e2[n, m]: rows = [den|v|den],
            # cols = 256 agents; one 256-col matmul per chunk.
            agvTp = ppsum.tile([128, 256], F32, tag="pB", name="agvTp")
            NQ = NCH // 4

            def ks_l2exp(qq):
                l2p = ppsum.tile([128, 1024], F32, tag="pA", name="l2p")
                for ci in range(4):
                    ssl = slice((4 * qq + ci) * 128, (4 * qq + ci + 1) * 128)
                    nc.tensor.matmul(l2p[:, ci * 256:(ci + 1) * 256],
                                     kfull[:, ssl], ag_full[:],
                                     start=True, stop=True)
                e2t = work.tile([128, 1024], F16, tag="e2t", name="e2t")
                nc.scalar.activation(e2t[:], l2p[:], AF.Exp)
                return e2t

            # software-pipelined: l2(q+1) is emitted before agvT(q) so the
            # in-order PE queue never stalls on the exp of the current quad
            e_prev = ks_l2exp(0)
            for qq in range(NQ):
                e_next = ks_l2exp(qq + 1) if qq + 1 < NQ else None
                for ci in range(4):
                    nc.tensor.matmul(agvTp[0:98, :],
                                     vT3[:, 4 * qq + ci, :],
                                     e_prev[:, ci * 256:(ci + 1) * 256],
                                     start=(qq == 0 and ci == 0),
                                     stop=(qq == NQ - 1 and ci == 3))
                e_prev = e_next

            # ====== av_l build: f16-convert agvT, transpose per head-pair =
            agvT_f16 = work1.tile([128, 256], F16, tag="agvf")
            nc.scalar.copy(agvT_f16[:], agvTp[:])
            avT0 = ppsum.tile([128, 128], F32, tag="pA", name="avT0")
            avT1 = ppsum.tile([128, 128], F32, tag="pB", name="avT1")
            nc.tensor.matmul(avT0[:, 0:98], agvT_f16[0:98, 0:128],
                             idmat[0:98, 0:98], start=True, stop=True)
            nc.tensor.matmul(avT1[:, 0:98], agvT_f16[0:98, 128:256],
                             idmat[0:98, 0:98], start=True, stop=True)
            # avT*: rows = agents, col 0 = denominator, cols 1:97 = v-ch 0:96
            nc.vector.reciprocal_approx_fast(out=rq2a[:], in_=avT0[:, 0:1])
            nc.vector.reciprocal_approx_fast(out=rq2b[:], in_=avT1[:, 0:1])
            # block-diagonal: even head of pair -> rows 0:64 x cols 0:24,
            # odd head -> rows 64:128 x cols 24:48 (other entries stay zero)
            nc.vector.tensor_scalar(out=av_l0[0:64, 0:24], in0=avT0[0:64, 1:25],
                                    scalar1=rq2a[0:64, :], scalar2=None, op0=OP.mult)
            for w0 in (64, 96):
                nc.vector.tensor_scalar(out=av_l0[w0:w0 + 32, 24:48],
                                        in0=avT0[w0:w0 + 32, 25:49],
                                        scalar1=rq2a[w0:w0 + 32, :], scalar2=None,
                                        op0=OP.mult)
            nc.vector.tensor_scalar(out=av_l1[0:64, 0:24], in0=avT1[0:64, 49:73],
                                    scalar1=rq2b[0:64, :], scalar2=None, op0=OP.mult)
            for w0 in (64, 96):
                nc.vector.tensor_scalar(out=av_l1[w0:w0 + 32, 24:48],
                                        in0=avT1[w0:w0 + 32, 73:97],
                                        scalar1=rq2b[w0:w0 + 32, :], scalar2=None,
                                        op0=OP.mult)

            if debug:
                nc.sync.dma_start(out=dbg_ag[:], in_=ag_full[:])
                nc.sync.dma_start(out=dbg_av0[:], in_=av_l0[:])
                nc.sync.dma_start(out=dbg_av1[:], in_=av_l1[:])
                nc.sync.dma_start(out=dbg_vt[:], in_=vT[:, 0:98 * 4])
            # ================= q-side + division =========================
            # Both head-pairs per j-block: op_/od_ psum rows 0:48 (hp0) and
            # 64:112 (hp1); one recip + one STT over [128, BLK2] covers both.
            # x_attn rows 48:64 / 112:128 are junk, skipped at output DMA.
            def qs_l1exp(j):
                e1s = []
                for hp in range(2):
                    ag_cols = ag_full[:, hp * 128:(hp + 1) * 128]
                    l1 = ppsum.tile([128, BLK2], F32, tag="pA", name="l1")
                    for q in range(2):
                        sl = slice(j * BLK2 + q * 512, j * BLK2 + (q + 1) * 512)
                        psl = slice(q * 512, (q + 1) * 512)
                        nc.tensor.matmul(l1[:, psl], ag_cols, dw0[0:96, sl],
                                         start=True, stop=True)
                    e1 = work.tile([128, BLK2], F16, tag=f"e1{hp}", name="e1")
                    nc.scalar.activation(e1[:], l1[:], AF.Exp)
                    e1s.append(e1)
                return e1s

            def qs_opod(j, e1s):
                blk = slice(j * BLK2, (j + 1) * BLK2)
                op_ = ppsum.tile([128, BLK2], F32, tag="pB", name="op_")
                od_ = ppsum.tile([128, BLK2], F32, tag="pB", name="od_")
                for hp in range(2):
                    rb = 64 * hp
                    av_l = av_l0 if hp == 0 else av_l1
                    for q in range(2):
                        psl = slice(q * 512, (q + 1) * 512)
                        nc.tensor.matmul(op_[rb:rb + 48, psl], av_l[:],
                                         e1s[hp][:, psl], start=True, stop=True)
                        nc.tensor.matmul(od_[rb:rb + 48, psl], dv_ones[:],
                                         e1s[hp][:, psl], start=True, stop=True)
                rqs = work1.tile([128, BLK2], F32, tag="rqs")
                nc.vector.reciprocal_approx_fast(out=rqs[:], in_=od_[:])
                nc.vector.scalar_tensor_tensor(
                    out=x_attn[:, blk], in0=op_[:], scalar=0.0,
                    in1=rqs[:], op0=OP.bypass, op1=OP.mult,
                    accum_out=mu_parts[:, j:j + 1])
                x2t = work.tile([128, BLK2], F16, tag="x2t")
                nc.scalar.activation(x2t[:], x_attn[:, blk], AF.Square,
                                     accum_out=x2_parts[:, j:j + 1])

            # software-pipelined like the k-side
            pend = qs_l1exp(0)
            for j in range(1, NB2):
                nxt = qs_l1exp(j)
                qs_opod(j - 1, pend)
                pend = nxt
            qs_opod(NB2 - 1, pend)

            if debug:
                nc.sync.dma_start(out=dbg_xa[:], in_=x_attn[:])
            # ================= SimAM =====================================
            # all [128, *]: rows 48:64 / 112:128 are junk lanes, skipped at
            # the output DMAs; per-partition stats keep junk contained.
            # sum(d2) = sum(x^2) - N*mu^2 (both accumulated in the q-side),
            # and s*(x-mu)^2 = (sqrt(s)*x - sqrt(s)*mu)^2 folds into one
            # Square activation, so no separate d2 pass over N is needed.
            nc.vector.reduce_sum(musum[:], mu_parts[:], axis=AX.X)
            nc.vector.reduce_sum(sx2[:], x2_parts[:], axis=AX.X)
            mu2 = work1.tile([128, 1], F32, tag="mu2")
            nc.vector.tensor_tensor(out=mu2[:], in0=musum[:], in1=musum[:],
                                    op=OP.mult)
            nc.vector.scalar_tensor_tensor(
                out=sden[:], in0=mu2[:], scalar=-1.0 / N, in1=sx2[:],
                op0=OP.mult, op1=OP.add)
            nc.vector.tensor_scalar(out=sden[:], in0=sden[:],
                                    scalar1=4.0 / (N - 1), scalar2=4.0 * E_LAMBDA,
                                    op0=OP.mult, op1=OP.add)
            nc.vector.reciprocal_approx_fast(out=s_ch[:], in_=sden[:])
            nc.scalar.activation(sqs[:], s_ch[:], AF.Sqrt)
            nc.vector.tensor_scalar(out=biasb[:], in0=musum[:], scalar1=sqs[:],
                                    scalar2=-1.0 / N, op0=OP.mult, op1=OP.mult)
            for j in range(NB2):
                blk = slice(j * BLK2, (j + 1) * BLK2)
                d2t = work.tile([128, BLK2], F16, tag="d2t")
                nc.scalar.activation(d2t[:], x_attn[:, blk], AF.Square,
                                     bias=biasb[:], scale=sqs[:])
                sig_t = work.tile([128, BLK2], F16, tag="sig_t")
                nc.scalar.activation(sig_t[:], d2t[:], AF.Sigmoid,
                                     bias=half_s[:])
                ob = pout.tile([128, BLK2], F32, tag="ob")
                nc.vector.tensor_tensor(out=ob[:], in0=x_attn[:, blk],
                                        in1=sig_t[:], op=OP.mult)
                nc.sync.dma_start(out=out_d[0:48, blk], in_=ob[0:48, :])
                nc.sync.dma_start(out=out_d[48:96, blk], in_=ob[64:112, :])

    nc.compile()
    return nc


_NC = None


def _get_nc():
    global _NC
    if _NC is None:
        _install_ntff_hook()
        _NC = build_nc()
    return _NC


def make_core_inputs(x, w_qkv, w_dw, temperature):
    """Host-side shard prep. Returns list of 8 input dicts."""
    x = np.asarray(x)
    w_qkv = np.asarray(w_qkv)
    w_dw = np.asarray(w_dw)
    temperature = np.asarray(temperature).reshape(8)
    in_maps = []
    for core in range(8):
        b, hg = core // 2, core % 2
        # slab0 = q + k[0:32]; slab1 = k[32:96] + a[0:64];
        # slab2 = v[0:96] + a[64:96]  (v at base 0 for PE transpose)
        rows = np.concatenate([
            np.arange(hg * 96, hg * 96 + 96),           # q
            192 + np.arange(hg * 96, hg * 96 + 96),     # k
            576 + np.arange(hg * 96, hg * 96 + 64),     # a[0:64]
            384 + np.arange(hg * 96, hg * 96 + 96),     # v
            576 + np.arange(hg * 96 + 64, hg * 96 + 96),  # a[64:96]
        ])
        W1 = w_qkv[rows, :, 0, 0]                        # [384, 192]
        W1T = np.ascontiguousarray(W1.T).astype(np.float16)
        wd9 = w_dw[rows, 0].reshape(384, 9).astype(np.float32)
        wdiag_h = np.zeros((128, NDIAG * 128), np.float16)
        wtap_h = np.zeros((128, 27), np.float32)
        for s in range(3):
            for t in range(9):
                wtap_h[:, s * 9 + t] = wd9[s * 128:(s + 1) * 128, t]
        for (s, dy, dx), idx in WDIAG_SLOT.items():
            t = (dy + 1) * 3 + (dx + 1)
            wdiag_h[np.arange(128), idx * 128 + np.arange(128)] = \
                wd9[s * 128:(s + 1) * 128, t].astype(np.float16)
        pat_h = np.zeros((128, 496), np.float16)
        pat_h[np.arange(128), 240 + np.arange(128)] = 1  # I128 for transposes
        for h in range(4):
            pat_h[h * 24:(h + 1) * 24, h * 24:(h + 1) * 24] = 1    # ones_q
        # D1-rep ones: cols 192:216 (rows 0:64), cols 216:240 (rows 64:128)
        pat_h[0:64, 192:216] = 1
        pat_h[64:128, 216:240] = 1
        # ones_kA/ones_kB at cols 368:496: pk output row m maps directly to
        # the k-channel partition homes: rows 0:64 -> k-ch 32+m (dw1),
        # rows 96:128 -> k-ch m-96 (dw0); rows 64:96 unused.
        hrow = np.full(128, -1)
        hrow[0:64] = (32 + np.arange(64)) // 24
        hrow[96:128] = np.arange(32) // 24
        pat_h[0:32, 368:496] = (
            (np.arange(32)[:, None] // 24) == hrow[None, :]).astype(np.float16)
        pat_h[32:96, 368:496] = (
            ((32 + np.arange(64))[:, None] // 24) == hrow[None, :]
        ).astype(np.float16)
        heads = np.arange(hg * 4, hg * 4 + 4)
        t4 = temperature[heads].astype(np.float32)
        in_maps.append({
            "xin": x[b].reshape(192, N).astype(np.float16),
            "w1a": W1T[0:96].copy(),
            "w1b": W1T[96:192].copy(),
            "wdiag": wdiag_h,
            "wtap": wtap_h,
            "tmp0": np.repeat(t4[0:2], 24).reshape(48, 1).copy(),
            "tmp1": np.repeat(t4[2:4], 24).reshape(48, 1).copy(),
            "pat": pat_h,
        })
    return in_maps


def _assemble(results):
    full = np.empty((B, C, H, W), np.float32)
    for core in range(8):
        b, hg = core // 2, core % 2
        full[b, hg * 96:(hg + 1) * 96] = results[core]["out"].reshape(96, H, W)
    return full


def kernel(x, w_qkv, w_dw, temperature):
    nc = _get_nc()
    in_maps = make_core_inputs(x, w_qkv, w_dw, temperature)
    res = run_bass_kernel_spmd(nc, in_maps, list(range(8)))
    return _assemble(res.results)


def kernel_profiled(x, w_qkv, w_dw, temperature):
    nc = _get_nc()
    in_maps = make_core_inputs(x, w_qkv, w_dw, temperature)
    res = run_bass_kernel_spmd(nc, in_maps, list(range(8)), trace=True)
    return _assemble(res.results), res.exec_time_ns



# revision 67
# speedup vs baseline: 2.6326x; 1.0024x over previous
"""MASA agent-attention kernel for Trainium2, 8-core SPMD.

Sharding: core = (batch b in 0..3) x (head-group hg in 0..1).
Each core computes conv1x1 + depthwise3x3 for its 4 heads' q/k/v/a
channels (384 of 768), the agent attention for those heads, and SimAM
over its 96 output channels. No cross-core communication.

Per-core channel order: [q(96), k(96), a(0:64), v(96), a(64:96)].
SBUF slabs of 128: s0 = q[0:96]+k[0:32], s1 = k[32:96]+a[0:64],
s2 = v[0:96]+a[64:96].  v at slab base 0 so the v-transpose is one
[96,128] PE matmul (vs identity) per 128-pixel chunk.

Engine-op partition windows must be 32-aligned and (base==0 or count<=32).
"""

import sys
import types
import numpy as np

import concourse.bacc as bacc
import concourse.bass as bass
import concourse.mybir as mybir
from concourse.tile import TileContext
from concourse.bass_utils import run_bass_kernel_spmd

F16 = mybir.dt.float16
F32 = mybir.dt.float32
AX = mybir.AxisListType
OP = mybir.AluOpType
AF = mybir.ActivationFunctionType

B, C, H, W = 4, 192, 128, 128
N = H * W              # 16384
M_AG = 64              # agent tokens
E_LAMBDA = 1e-4
RS = 130               # padded row stride for pre
PREFREE = RS * RS      # 16900

TAPS = [(dy, dx) for dy in (-1, 0, 1) for dx in (-1, 0, 1)]
# tap offset in pre: (1+dy)*RS + (1+dx); odd offsets (dx==0) are
# 4B-misaligned for fp16 2x mode -> always on PE. DVE gets only the
# dx=+1 column (aligned), as tensor_scalar products + tensor_tensor
# adds (packed modes); scalar_tensor_tensor is always 1x on DVE.
PE_TAPS = {
    0: TAPS,                                  # slab0 fully on PE
    1: [t for t in TAPS if t[1] <= 0],        # dx in {-1, 0}
    2: [t for t in TAPS if t[1] <= 0],
}
DVE_TAPS = {s: [t for t in TAPS if t not in PE_TAPS[s]] for s in range(3)}
WDIAG_SLOT = {}
for _s in range(3):
    for _t in PE_TAPS[_s]:
        WDIAG_SLOT[(_s, _t[0], _t[1])] = len(WDIAG_SLOT)
NDIAG = len(WDIAG_SLOT)

NB2 = 16               # block count for norm / attention / simam phases
BLK2 = 1024
NCH = 128              # s-chunks of 128 for k-side


def _install_ntff_hook():
    try:
        import antenv.axon_hooks  # noqa: F401
        return
    except ImportError:
        pass
    try:
        from trn_agent_boot.trn_boot import _ntff_profile_via_ctypes
        hook = _ntff_profile_via_ctypes('/opt/axon/libaxon_pjrt.so')
        mod = types.ModuleType("antenv.axon_hooks")
        mod.get_axon_ntff_profile_hook = lambda: hook
        mod.set_axon_ntff_profile_hook = lambda h: None
        sys.modules["antenv.axon_hooks"] = mod
    except Exception:
        pass


def build_nc(debug=False):
    nc = bacc.Bacc("TRN2", target_bir_lowering=False, debug=False, num_devices=8)

    # ---- DRAM I/O ----
    xin = nc.dram_tensor("xin", [192, N], F16, kind="ExternalInput").ap()
    w1a = nc.dram_tensor("w1a", [96, 384], F16, kind="ExternalInput").ap()
    w1b = nc.dram_tensor("w1b", [96, 384], F16, kind="ExternalInput").ap()
    wdiag = nc.dram_tensor("wdiag", [128, NDIAG * 128], F16, kind="ExternalInput").ap()
    wtap = nc.dram_tensor("wtap", [128, 27], F32, kind="ExternalInput").ap()
    tmp0 = nc.dram_tensor("tmp0", [48, 1], F32, kind="ExternalInput").ap()
    tmp1 = nc.dram_tensor("tmp1", [48, 1], F32, kind="ExternalInput").ap()
    pat = nc.dram_tensor("pat", [128, 496], F16, kind="ExternalInput").ap()
    out_d = nc.dram_tensor("out", [96, N], F32, kind="ExternalOutput").ap()
    if debug:
        dbg_pre = nc.dram_tensor("dbg_pre", [128, PREFREE], F16, kind="ExternalOutput").ap()
        dbg_q = nc.dram_tensor("dbg_q", [128, N], F16, kind="ExternalOutput").ap()
        dbg_k = nc.dram_tensor("dbg_k", [128, N], F16, kind="ExternalOutput").ap()
        dbg_qn = nc.dram_tensor("dbg_qn", [128, N], F16, kind="ExternalOutput").ap()
        dbg_ag = nc.dram_tensor("dbg_ag", [96, 256], F16, kind="ExternalOutput").ap()
        dbg_av0 = nc.dram_tensor("dbg_av0", [128, 48], F16, kind="ExternalOutput").ap()
        dbg_av1 = nc.dram_tensor("dbg_av1", [128, 48], F16, kind="ExternalOutput").ap()
        dbg_xa = nc.dram_tensor("dbg_xa", [128, N], F16, kind="ExternalOutput").ap()
        dbg_vt = nc.dram_tensor("dbg_vt", [128, 98 * 4], F16, kind="ExternalOutput").ap()
        dbg_avi = nc.dram_tensor("dbg_avi", [128, 48], F16, kind="ExternalOutput").ap()
        dbg_e1 = nc.dram_tensor("dbg_e1", [128, BLK2], F16, kind="ExternalOutput").ap()
        dbg_op = nc.dram_tensor("dbg_op", [128, BLK2], F32, kind="ExternalOutput").ap()
        dbg_rqs = nc.dram_tensor("dbg_rqs", [48, BLK2], F32, kind="ExternalOutput").ap()

    # ---- persistent SBUF ----
    scratch = nc.alloc_sbuf_tensor("scratch", [128, PREFREE], F16).ap()
    dw0 = nc.alloc_sbuf_tensor("dw0", [128, N], F16).ap()
    dw1 = nc.alloc_sbuf_tensor("dw1", [128, N], F16).ap()
    dw2 = nc.alloc_sbuf_tensor("dw2", [128, N], F16).ap()
    dws = [dw0, dw1, dw2]
    w1a_s = nc.alloc_sbuf_tensor("w1a_s", [96, 384], F16).ap()
    w1b_s = nc.alloc_sbuf_tensor("w1b_s", [96, 384], F16).ap()
    wdiag_s = nc.alloc_sbuf_tensor("wdiag_s", [128, NDIAG * 128], F16).ap()
    wtap_s = nc.alloc_sbuf_tensor("wtap_s", [128, 27], F32).ap()
    ones_q = nc.alloc_sbuf_tensor("ones_q", [96, 96], F16).ap()
    ones_kA = nc.alloc_sbuf_tensor("ones_kA", [32, 128], F16).ap()
    ones_kB = nc.alloc_sbuf_tensor("ones_kB", [64, 128], F16).ap()
    ag_full = nc.alloc_sbuf_tensor("ag_full", [96, 256], F16).ap()
    agf = nc.alloc_sbuf_tensor("agf", [96, M_AG], F32).ap()
    agfs = nc.alloc_sbuf_tensor("agfs", [96, M_AG], F16).ap()
    temp_rep = nc.alloc_sbuf_tensor("temp_rep", [96, 1], F32).ap()
    av_l0 = nc.alloc_sbuf_tensor("av_l0", [128, 48], F16).ap()
    av_l1 = nc.alloc_sbuf_tensor("av_l1", [128, 48], F16).ap()
    dv_ones = nc.alloc_sbuf_tensor("dv_ones", [128, 48], F16).ap()
    idmat = nc.alloc_sbuf_tensor("idmat", [128, 128], F16).ap()
    asum = nc.alloc_sbuf_tensor("asum", [128, 2 * M_AG], F32).ap()  # rows 64:128
    as1_t = nc.alloc_sbuf_tensor("as1_t", [128, 1024], F32).ap()    # pool stage1
    rq2a = nc.alloc_sbuf_tensor("rq2a", [128, 1], F32).ap()
    rq2b = nc.alloc_sbuf_tensor("rq2b", [128, 1], F32).ap()
    mu_parts = nc.alloc_sbuf_tensor("mu_parts", [128, NB2], F32).ap()
    x2_parts = nc.alloc_sbuf_tensor("x2_parts", [128, NB2], F32).ap()
    musum = nc.alloc_sbuf_tensor("musum", [128, 1], F32).ap()
    sx2 = nc.alloc_sbuf_tensor("sx2", [128, 1], F32).ap()
    sden = nc.alloc_sbuf_tensor("sden", [128, 1], F32).ap()
    s_ch = nc.alloc_sbuf_tensor("s_ch", [128, 1], F32).ap()
    sqs = nc.alloc_sbuf_tensor("sqs", [128, 1], F32).ap()
    biasb = nc.alloc_sbuf_tensor("biasb", [128, 1], F32).ap()
    half_s = nc.alloc_sbuf_tensor("half_s", [128, 1], F32).ap()

    # aliases (sequential reuse of big buffers)
    pre3 = scratch.rearrange("p (y x) -> p y x", x=RS)   # padded conv out
    vT = scratch[:, 0:NCH * 98]                          # after dwconv
    x_attn = dw1[:, :]                                   # [128, N] f16 (phase D)
    kfull = dw2[0:96, :]                                 # k-hat packed (phase B)

    with TileContext(nc) as tc:
        with (
            tc.tile_pool(name="xio", bufs=4) as xio,
            tc.tile_pool(name="pout", bufs=2) as pout,
            tc.tile_pool(name="work", bufs=2) as work,
            tc.tile_pool(name="work1", bufs=1) as work1,
            tc.tile_pool(name="ppsum", bufs=2, space="PSUM") as ppsum,
        ):
            # ================= init =================
            nc.sync.dma_start(out=w1a_s[:], in_=w1a[:])
            nc.sync.dma_start(out=w1b_s[:], in_=w1b[:])
            nc.sync.dma_start(out=wdiag_s[:], in_=wdiag[:])
            nc.sync.dma_start(out=wtap_s[:], in_=wtap[:])
            # static patterns
            nc.sync.dma_start(out=ones_q[:], in_=pat[0:96, 0:96])
            nc.sync.dma_start(out=ones_kA[:], in_=pat[0:32, 368:496])
            nc.sync.dma_start(out=ones_kB[:], in_=pat[32:96, 368:496])
            nc.gpsimd.memset(av_l0[:], 0.0)
            nc.gpsimd.memset(av_l1[:], 0.0)
            # D1-rep ones lhsT: col j<24 -> even head (rows 0:64),
            # j>=24 -> odd head (rows 64:128)
            nc.sync.dma_start(out=dv_ones[:, 0:24], in_=pat[:, 192:216])
            nc.sync.dma_start(out=dv_ones[:, 24:48], in_=pat[:, 216:240])
            nc.sync.dma_start(out=idmat[:], in_=pat[:, 240:368])
            nc.gpsimd.memset(ag_full[:], 0.0)
            nc.sync.dma_start(out=temp_rep[0:48, :], in_=tmp0[:])
            nc.sync.dma_start(out=temp_rep[48:96, :], in_=tmp1[:])
            nc.gpsimd.memset(half_s[:], 0.5)
            # pre borders (rows 0 and 129, cols 0 and 129)
            nc.gpsimd.memset(pre3[:, 0, :], 0.0)
            nc.gpsimd.memset(pre3[:, 129, :], 0.0)
            nc.gpsimd.memset(pre3[:, :, 0], 0.0)
            nc.gpsimd.memset(pre3[:, :, 129], 0.0)

            if debug:
                nc.sync.dma_start(out=dbg_avi[:], in_=dv_ones[:])
            # ================= sweep1: conv1x1 + dwconv ====
            for s in range(3):
                wa = w1a_s[:, s * 128:(s + 1) * 128]
                wb = w1b_s[:, s * 128:(s + 1) * 128]
                nblk = N // 1024  # 16 blocks of 1024 (8 y-rows)

                def conv_blk(j, s=s, wa=wa, wb=wb):
                    x0 = xio.tile([96, 1024], F16, tag="x")
                    x1 = xio.tile([96, 1024], F16, tag="x")
                    nc.sync.dma_start(out=x0[:], in_=xin[0:96, j * 1024:(j + 1) * 1024])
                    nc.sync.dma_start(out=x1[:], in_=xin[96:192, j * 1024:(j + 1) * 1024])
                    ps = ppsum.tile([128, 1024], F32, tag="pA")
                    for q in range(2):
                        sl = slice(q * 512, (q + 1) * 512)
                        nc.tensor.matmul(ps[:, sl], wa, x0[:, sl], start=True, stop=False)
                        nc.tensor.matmul(ps[:, sl], wb, x1[:, sl], start=False, stop=True)
                    nc.scalar.copy(pre3[:, 1 + 8 * j: 9 + 8 * j, 1:129], ps[:])

                def dw_blk(j, s=s):
                    dst = dws[s][:, j * 1024:(j + 1) * 1024]
                    pe_t = PE_TAPS[s]
                    dv_t = DVE_TAPS[s]
                    pd = None
                    if pe_t:
                        pd = ppsum.tile([128, 1024], F32, tag="pB")
                        for q in range(2):
                            for ti, (dy, dx) in enumerate(pe_t):
                                dg = wdiag_s[:, WDIAG_SLOT[(s, dy, dx)] * 128:
                                             (WDIAG_SLOT[(s, dy, dx)] + 1) * 128]
                                rv = pre3[:, 1 + dy + 8 * j + 4 * q: 5 + dy + 8 * j + 4 * q,
                                          1 + dx: 129 + dx]
                                nc.tensor.matmul(pd[:, q * 512:(q + 1) * 512], dg, rv,
                                                 start=(ti == 0), stop=(ti == len(pe_t) - 1))
                    if dv_t:
                        # 3 aligned taps: STT on DVE (merges PE psum, 1x);
                        # the other 2 products on gpsimd (idle in sweep1),
                        # summed into dst by 2 DVE TT adds.
                        def win(dy, dx):
                            return pre3[:, 1 + dy + 8 * j: 9 + dy + 8 * j,
                                        1 + dx: 129 + dx]

                        def wsc(dy, dx):
                            ti = s * 9 + TAPS.index((dy, dx))
                            return wtap_s[:, ti:ti + 1]

                        ta = work.tile([128, 1024], F16, tag="dta")
                        nc.vector.scalar_tensor_tensor(
                            out=ta[:], in0=win(*dv_t[0]), scalar=wsc(*dv_t[0]),
                            in1=pd[:], op0=OP.mult, op1=OP.add)
                        tb = work.tile([128, 1024], F16, tag="dtb")
                        nc.vector.tensor_scalar(
                            out=tb[:], in0=win(*dv_t[1]), scalar1=wsc(*dv_t[1]),
                            scalar2=None, op0=OP.mult)
                        nc.vector.tensor_scalar(
                            out=dst, in0=win(*dv_t[2]), scalar1=wsc(*dv_t[2]),
                            scalar2=None, op0=OP.mult)
                        nc.vector.tensor_tensor(out=dst, in0=ta[:], in1=dst,
                                                op=OP.add)
                        nc.vector.tensor_tensor(out=dst, in0=tb[:], in1=dst,
                                                op=OP.add)
                    else:
                        nc.scalar.copy(dst, pd[:])

                conv_blk(0)
                for j in range(1, nblk):
                    conv_blk(j)
                    dw_blk(j - 1)
                dw_blk(nblk - 1)

                # pooling (both stages), emitted right after the slab that
                # produces its a-rows (as1_t is a dedicated buffer, so no
                # false dependency on the pre3 scratch region; rows 96:128
                # of as1_t are reused sequentially by the 2nd and 3rd group)
                if s == 1:
                    pgroups = ((dw1, 64, 0), (dw1, 96, 0))
                elif s == 2:
                    pgroups = ((dw2, 96, 1),)
                else:
                    pgroups = ()
                for (abuf, w0, half) in pgroups:
                    a3 = abuf[w0:w0 + 32, :].rearrange("p (a xi) -> p a xi",
                                                       xi=16)
                    s1 = as1_t[w0:w0 + 32, :]
                    nc.vector.reduce_sum(s1, a3, axis=AX.X)
                    as3 = s1.rearrange("p (yb yi xb) -> p yb xb yi",
                                       yb=8, yi=16, xb=8)
                    asum3 = asum[w0:w0 + 32,
                                 half * 64:(half + 1) * 64].rearrange(
                        "p (yb xb) -> p yb xb", yb=8)
                    nc.vector.reduce_sum(asum3, as3, axis=AX.X)

            if debug:
                nc.sync.dma_start(out=dbg_pre[:], in_=scratch[:])
                nc.sync.dma_start(out=dbg_q[:], in_=dw0[:])
                nc.sync.dma_start(out=dbg_k[:], in_=dw1[:])
            nc.sync.dma_start(out=agf[0:32, :], in_=asum[64:96, 0:64])
            nc.sync.dma_start(out=agf[32:64, :], in_=asum[96:128, 0:64])
            nc.sync.dma_start(out=agf[64:96, :], in_=asum[96:128, 64:128])
            # scale by temp/256 (per-partition scalar), then place blocks by DMA
            nc.vector.tensor_scalar(out=agfs[:], in0=agf[:],
                                    scalar1=temp_rep[:], scalar2=1.0 / 256.0,
                                    op0=OP.mult, op1=OP.mult)
            for h in range(4):
                nc.sync.dma_start(
                    out=ag_full[h * 24:(h + 1) * 24, h * 64:(h + 1) * 64],
                    in_=agfs[h * 24:(h + 1) * 24, :])

            # vT ones (denominator) columns; gpsimd queue is otherwise empty
            # here so these run as soon as the pre3 readers finish
            vT3 = vT.rearrange("p (c w) -> p c w", w=98)
            nc.gpsimd.memset(vT3[:, :, 0], 1.0)
            nc.gpsimd.memset(vT3[:, :, 97], 1.0)

            # ====== merged middle: per j: l2norm + vT group + k-side =====
            # ====== vT build: one dense PE burst =========================
            # fills the PE hole while pooling / sweep-tail drain the DVE.
            # Runs BEFORE any kpack DMA: kfull aliases dw2[0:96], so packing
            # k-hat destroys v.  (4 chunks per 2KB PSUM bank: a matmul
            # output must not cross a bank boundary.)
            for j in range(NB2):
                pt = ppsum.tile([128, 1024], F32, tag="pA", name="pt")
                for ci in range(8):
                    ssl = slice((8 * j + ci) * 128, (8 * j + ci + 1) * 128)
                    off = 512 * (ci // 4) + 96 * (ci % 4)
                    nc.tensor.matmul(pt[:, off:off + 96],
                                     dw2[0:96, ssl], idmat[0:96, 0:96],
                                     start=True, stop=True)
                pt3 = pt.rearrange("p (b x) -> p b x", b=2)
                nc.scalar.copy(vT3[:, 8 * j:8 * j + 8, 1:97],
                               pt3[:, :, 0:384])

            # keeps the PE queue dense through this region (HAM stays warm)
            for j in range(NB2):
                blk = slice(j * BLK2, (j + 1) * BLK2)
                sq0 = work1.tile([128, BLK2], F16, tag="sq0", bufs=2)
                sq1 = work1.tile([64, BLK2], F16, tag="sq1")
                sqk = work1.tile([32, BLK2], F16, tag="sqk")
                nc.gpsimd.tensor_tensor(out=sq0[:], in0=dw0[:, blk], in1=dw0[:, blk],
                                        op=OP.mult)
                nc.scalar.activation(sq1[:], dw1[0:64, blk], AF.Square)
                nc.sync.dma_start(out=sqk[:], in_=sq0[96:128, :])
                pq = ppsum.tile([96, BLK2], F32, tag="pA")
                pk = ppsum.tile([128, BLK2], F32, tag="pB")
                for q in range(2):
                    sl = slice(q * 512, (q + 1) * 512)
                    nc.tensor.matmul(pq[:, sl], ones_q[:], sq0[0:96, sl],
                                     start=True, stop=True)
                    nc.tensor.matmul(pk[:, sl], ones_kA[:], sqk[:, sl],
                                     start=True, stop=False)
                    nc.tensor.matmul(pk[:, sl], ones_kB[:], sq1[:, sl],
                                     start=False, stop=True)
                rinv_q = work1.tile([96, BLK2], F16, tag="rinv_q")
                rinv_k = work1.tile([128, BLK2], F16, tag="rinv_k")
                nc.scalar.activation(rinv_q[:], pq[:], AF.Abs_reciprocal_sqrt)
                nc.scalar.activation(rinv_k[:], pk[:], AF.Abs_reciprocal_sqrt)
                nc.vector.tensor_tensor(out=dw0[0:96, blk], in0=dw0[0:96, blk],
                                        in1=rinv_q[:], op=OP.mult)
                nc.vector.tensor_tensor(out=dw0[96:128, blk], in0=dw0[96:128, blk],
                                        in1=rinv_k[96:128, :], op=OP.mult)
                nc.gpsimd.tensor_tensor(out=dw1[0:64, blk], in0=dw1[0:64, blk],
                                        in1=rinv_k[0:64, :], op=OP.mult)
                nc.sync.dma_start(out=kfull[0:32, blk], in_=dw0[96:128, blk])
                nc.sync.dma_start(out=kfull[32:96, blk], in_=dw1[0:64, blk])
            if debug:
                nc.sync.dma_start(out=dbg_qn[:], in_=dw0[:])

            # ====== k-side: l2 -> exp -> agvT accumulate (dense PE loop) ==
            # agvT[r, m] = sum_n vT3[n, r] * e2[n, m]: rows = [den|v|den],
            # cols = 256 agents; one 256-col matmul per chunk.
            agvTp = ppsum.tile([128, 256], F32, tag="pB", name="agvTp")
            NQ = NCH // 4

            def ks_l2exp(qq):
                l2p = ppsum.tile([128, 1024], F32, tag="pA", name="l2p")
                for ci in range(4):
                    ssl = slice((4 * qq + ci) * 128, (4 * qq + ci + 1) * 128)
                    nc.tensor.matmul(l2p[:, ci * 256:(ci + 1) * 256],
                                     kfull[:, ssl], ag_full[:],
                                     start=True, stop=True)
                e2t = work.tile([128, 1024], F16, tag="e2t", name="e2t")
                nc.scalar.activation(e2t[:], l2p[:], AF.Exp)
                return e2t

            # software-pipelined: l2(q+1) is emitted before agvT(q) so the
            # in-order PE queue never stalls on the exp of the current quad
            e_prev = ks_l2exp(0)
            for qq in range(NQ):
                e_next = ks_l2exp(qq + 1) if qq + 1 < NQ else None
                for ci in range(4):
                    nc.tensor.matmul(agvTp[0:98, :],
                                     vT3[:, 4 * qq + ci, :],
                                     e_prev[:, ci * 256:(ci + 1) * 256],
                                     start=(qq == 0 and ci == 0),
                                     stop=(qq == NQ - 1 and ci == 3))
                e_prev = e_next

            # ====== av_l build: f16-convert agvT, transpose per head-pair =
            agvT_f16 = work1.tile([128, 256], F16, tag="agvf")
            nc.scalar.copy(agvT_f16[:], agvTp[:])
            avT0 = ppsum.tile([128, 128], F32, tag="pA", name="avT0")
            avT1 = ppsum.tile([128, 128], F32, tag="pB", name="avT1")
            nc.tensor.matmul(avT0[:, 0:98], agvT_f16[0:98, 0:128],
                             idmat[0:98, 0:98], start=True, stop=True)
            nc.tensor.matmul(avT1[:, 0:98], agvT_f16[0:98, 128:256],
                             idmat[0:98, 0:98], start=True, stop=True)
            # avT*: rows = agents, col 0 = denominator, cols 1:97 = v-ch 0:96
            nc.vector.reciprocal_approx_fast(out=rq2a[:], in_=avT0[:, 0:1])
            nc.vector.reciprocal_approx_fast(out=rq2b[:], in_=avT1[:, 0:1])
            # block-diagonal: even head of pair -> rows 0:64 x cols 0:24,
            # odd head -> rows 64:128 x cols 24:48 (other entries stay zero)
            nc.vector.tensor_scalar(out=av_l0[0:64, 0:24], in0=avT0[0:64, 1:25],
                                    scalar1=rq2a[0:64, :], scalar2=None, op0=OP.mult)
            for w0 in (64, 96):
                nc.vector.tensor_scalar(out=av_l0[w0:w0 + 32, 24:48],
                                        in0=avT0[w0:w0 + 32, 25:49],
                                        scalar1=rq2a[w0:w0 + 32, :], scalar2=None,
                                        op0=OP.mult)
            nc.vector.tensor_scalar(out=av_l1[0:64, 0:24], in0=avT1[0:64, 49:73],
                                    scalar1=rq2b[0:64, :], scalar2=None, op0=OP.mult)
            for w0 in (64, 96):
                nc.vector.tensor_scalar(out=av_l1[w0:w0 + 32, 24:48],
                                        in0=avT1[w0:w0 + 32, 73:97],
                                        scalar1=rq2b[w0:w0 + 32, :], scalar2=None,
                                        op0=OP.mult)

            if debug:
                nc.sync.dma_start(out=dbg_ag[:], in_=ag_full[:])
                nc.sync.dma_start(out=dbg_av0[:], in_=av_l0[:])
                nc.sync.dma_start(out=dbg_av1[:], in_=av_l1[:])
                nc.sync.dma_start(out=dbg_vt[:], in_=vT[:, 0:98 * 4])
            # ================= q-side + division =========================
            # Both head-pairs per j-block: op_/od_ psum rows 0:48 (hp0) and
            # 64:112 (hp1); one recip + one STT over [128, BLK2] covers both.
            # x_attn rows 48:64 / 112:128 are junk, skipped at output DMA.
            def qs_l1exp(j):
                e1s = []
                for hp in range(2):
                    ag_cols = ag_full[:, hp * 128:(hp + 1) * 128]
                    l1 = ppsum.tile([128, BLK2], F32, tag="pA", name="l1")
                    for q in range(2):
                        sl = slice(j * BLK2 + q * 512, j * BLK2 + (q + 1) * 512)
                        psl = slice(q * 512, (q + 1) * 512)
                        nc.tensor.matmul(l1[:, psl], ag_cols, dw0[0:96, sl],
                                         start=True, stop=True)
                    e1 = work.tile([128, BLK2], F16, tag=f"e1{hp}", name="e1")
                    nc.scalar.activation(e1[:], l1[:], AF.Exp)
                    e1s.append(e1)
                return e1s

            def qs_opod(j, e1s):
                blk = slice(j * BLK2, (j + 1) * BLK2)
                op_ = ppsum.tile([128, BLK2], F32, tag="pB", name="op_")
                od_ = ppsum.tile([128, BLK2], F32, tag="pB", name="od_")
                for hp in range(2):
                    rb = 64 * hp
                    av_l = av_l0 if hp == 0 else av_l1
                    for q in range(2):
                        psl = slice(q * 512, (q + 1) * 512)
                        nc.tensor.matmul(op_[rb:rb + 48, psl], av_l[:],
                                         e1s[hp][:, psl], start=True, stop=True)
                        nc.tensor.matmul(od_[rb:rb + 48, psl], dv_ones[:],
                                         e1s[hp][:, psl], start=True, stop=True)
                rqs = work1.tile([128, BLK2], F32, tag="rqs")
                nc.vector.reciprocal_approx_fast(out=rqs[:], in_=od_[:])
                nc.vector.scalar_tensor_tensor(
                    out=x_attn[:, blk], in0=op_[:], scalar=0.0,
                    in1=rqs[:], op0=OP.bypass, op1=OP.mult,
                    accum_out=mu_parts[:, j:j + 1])
                x2t = work.tile([128, BLK2], F16, tag="x2t")
                nc.scalar.activation(x2t[:], x_attn[:, blk], AF.Square,
                                     accum_out=x2_parts[:, j:j + 1])

            # software-pipelined like the k-side
            pend = qs_l1exp(0)
            for j in range(1, NB2):
                nxt = qs_l1exp(j)
                qs_opod(j - 1, pend)
                pend = nxt
            qs_opod(NB2 - 1, pend)

            if debug:
                nc.sync.dma_start(out=dbg_xa[:], in_=x_attn[:])
            # ================= SimAM =====================================
            # all [128, *]: rows 48:64 / 112:128 are junk lanes, skipped at
            # the output DMAs; per-partition stats keep junk contained.
            # sum(d2) = sum(x^2) - N*mu^2 (both accumulated in the q-side),
            # and s*(x-mu)^2 = (sqrt(s)*x - sqrt(s)*mu)^2 folds into one
            # Square activation, so no separate d2 pass over N is needed.
            nc.vector.reduce_sum(musum[:], mu_parts[:], axis=AX.X)
            nc.vector.reduce_sum(sx2[:], x2_parts[:], axis=AX.X)
            mu2 = work1.tile([128, 1], F32, tag="mu2")
            nc.vector.tensor_tensor(out=mu2[:], in0=musum[:], in1=musum[:],
                                    op=OP.mult)
            nc.vector.scalar_tensor_tensor(
                out=sden[:], in0=mu2[:], scalar=-1.0 / N, in1=sx2[:],
                op0=OP.mult, op1=OP.add)
            nc.vector.tensor_scalar(out=sden[:], in0=sden[:],
                                    scalar1=4.0 / (N - 1), scalar2=4.0 * E_LAMBDA,
                                    op0=OP.mult, op1=OP.add)
            nc.vector.reciprocal_approx_fast(out=s_ch[:], in_=sden[:])
            nc.scalar.activation(sqs[:], s_ch[:], AF.Sqrt)
            nc.vector.tensor_scalar(out=biasb[:], in0=musum[:], scalar1=sqs[:],
                                    scalar2=-1.0 / N, op0=OP.mult, op1=OP.mult)
            for j in range(NB2):
                blk = slice(j * BLK2, (j + 1) * BLK2)
                d2t = work.tile([128, BLK2], F16, tag="d2t")
                nc.scalar.activation(d2t[:], x_attn[:, blk], AF.Square,
                                     bias=biasb[:], scale=sqs[:])
                sig_t = work.tile([128, BLK2], F16, tag="sig_t")
                nc.scalar.activation(sig_t[:], d2t[:], AF.Sigmoid,
                                     bias=half_s[:])
                ob = pout.tile([128, BLK2], F32, tag="ob")
                nc.vector.tensor_tensor(out=ob[:], in0=x_attn[:, blk],
                                        in1=sig_t[:], op=OP.mult)
                nc.sync.dma_start(out=out_d[0:48, blk], in_=ob[0:48, :])
                nc.sync.dma_start(out=out_d[48:96, blk], in_=ob[64:112, :])

    nc.compile()
    return nc


_NC = None


def _get_nc():
    global _NC
    if _NC is None:
        _install_ntff_hook()
        _NC = build_nc()
    return _NC


def make_core_inputs(x, w_qkv, w_dw, temperature):
    """Host-side shard prep. Returns list of 8 input dicts."""
    x = np.asarray(x)
    w_qkv = np.asarray(w_qkv)
    w_dw = np.asarray(w_dw)
    temperature = np.asarray(temperature).reshape(8)
    in_maps = []
    for core in range(8):
        b, hg = core // 2, core % 2
        # slab0 = q + k[0:32]; slab1 = k[32:96] + a[0:64];
        # slab2 = v[0:96] + a[64:96]  (v at base 0 for PE transpose)
        rows = np.concatenate([
            np.arange(hg * 96, hg * 96 + 96),           # q
            192 + np.arange(hg * 96, hg * 96 + 96),     # k
            576 + np.arange(hg * 96, hg * 96 + 64),     # a[0:64]
            384 + np.arange(hg * 96, hg * 96 + 96),     # v
            576 + np.arange(hg * 96 + 64, hg * 96 + 96),  # a[64:96]
        ])
        W1 = w_qkv[rows, :, 0, 0]                        # [384, 192]
        W1T = np.ascontiguousarray(W1.T).astype(np.float16)
        wd9 = w_dw[rows, 0].reshape(384, 9).astype(np.float32)
        wdiag_h = np.zeros((128, NDIAG * 128), np.float16)
        wtap_h = np.zeros((128, 27), np.float32)
        for s in range(3):
            for t in range(9):
                wtap_h[:, s * 9 + t] = wd9[s * 128:(s + 1) * 128, t]
        for (s, dy, dx), idx in WDIAG_SLOT.items():
            t = (dy + 1) * 3 + (dx + 1)
            wdiag_h[np.arange(128), idx * 128 + np.arange(128)] = \
                wd9[s * 128:(s + 1) * 128, t].astype(np.float16)
        pat_h = np.zeros((128, 496), np.float16)
        pat_h[np.arange(128), 240 + np.arange(128)] = 1  # I128 for transposes
        for h in range(4):
            pat_h[h * 24:(h + 1) * 24, h * 24:(h + 1) * 24] = 1    # ones_q
        # D1-rep ones: cols 192:216 (rows 0:64), cols 216:240 (rows 64:128)
        pat_h[0:64, 192:216] = 1
        pat_h[64:128, 216:240] = 1
        # ones_kA/ones_kB at cols 368:496: pk output row m maps directly to
        # the k-channel partition homes: rows 0:64 -> k-ch 32+m (dw1),
        # rows 96:128 -> k-ch m-96 (dw0); rows 64:96 unused.
        hrow = np.full(128, -1)
        hrow[0:64] = (32 + np.arange(64)) // 24
        hrow[96:128] = np.arange(32) // 24
        pat_h[0:32, 368:496] = (
            (np.arange(32)[:, None] // 24) == hrow[None, :]).astype(np.float16)
        pat_h[32:96, 368:496] = (
            ((32 + np.arange(64))[:, None] // 24) == hrow[None, :]
        ).astype(np.float16)
        heads = np.arange(hg * 4, hg * 4 + 4)
        t4 = temperature[heads].astype(np.float32)
        in_maps.append({
            "xin": x[b].reshape(192, N).astype(np.float16),
            "w1a": W1T[0:96].copy(),
            "w1b": W1T[96:192].copy(),
            "wdiag": wdiag_h,
            "wtap": wtap_h,
            "tmp0": np.repeat(t4[0:2], 24).reshape(48, 1).copy(),
            "tmp1": np.repeat(t4[2:4], 24).reshape(48, 1).copy(),
            "pat": pat_h,
        })
    return in_maps


def _assemble(results):
    full = np.empty((B, C, H, W), np.float32)
    for core in range(8):
        b, hg = core // 2, core % 2
        full[b, hg * 96:(hg + 1) * 96] = results[core]["out"].reshape(96, H, W)
    return full


def kernel(x, w_qkv, w_dw, temperature):
    nc = _get_nc()
    in_maps = make_core_inputs(x, w_qkv, w_dw, temperature)
    res = run_bass_kernel_spmd(nc, in_maps, list(range(8)))
    return _assemble(res.results)


def kernel_profiled(x, w_qkv, w_dw, temperature):
    nc = _get_nc()
    in_maps = make_core_inputs(x, w_qkv, w_dw, temperature)
    res = run_bass_kernel_spmd(nc, in_maps, list(range(8)), trace=True)
    return _assemble(res.results), res.exec_time_ns

